# revision 15
# baseline (speedup 1.0000x reference)
"""FootballGCN (3x GCNConv + mean-pool + MLP) on 8 TRN2 NeuronCores.

Self-contained: takes full inputs, shards internally, runs a Bass/Tile SPMD
kernel via run_bass_kernel_spmd, returns the full (B, 1) output.

Strategy (dst-sharded message passing, feature-major on-chip layout):
  - nodes packed into 8 cores x 12800 local slots (12500 real + pad)
  - per layer: table t = dis * (h @ W) built per 128-node block via one
    matmul (lhsT = feature-major h block -> node-major psum, no transpose),
    quantized f16, duplicated to 256B rows, staged per quarter and
    AllGathered slab-by-slab (4 pipelined collectives) to every core's HBM
  - edge pass: per superblock (SB dst blocks) x 4 source slabs (int16 index
    range; slab q is Q7-pair-q's SWDGE queue so desc-gen overlaps 4-way),
    dma_gather 256B rows with per-bucket trailing trim via num_idxs_reg;
    selection matrix built on DVE via is_equal(dst_local, iota); PE matmul
    (lhsT=msg, rhs=Sel) accumulates per-block agg[64, 128] in PSUM, seeded
    with the self-loop term t^T @ I; epilogue relu(dis*agg + b) on DVE+ACT
  - pooling via segsel matmuls, AllReduce, tiny MLP, output z
"""
import numpy as np

import concourse.bass as bass
import concourse.mybir as mybir
import concourse.tile as tile
from concourse import bacc as bacc_mod
from concourse.bass_utils import run_bass_kernel_spmd

F16 = mybir.dt.float16
F32 = mybir.dt.float32
I16 = mybir.dt.int16

# ---- problem dims (hardcoded per spec) ----
N = 100000
E = 3200000
B = 128
IN_C, HID = 128, 64
NCORES = 8
NREAL = 12500
NBLK = 100                   # blocks per core (multiple of 4 for slab split)
NLOC = NBLK * 128            # 12800
NTOT = NCORES * NLOC         # 102400
NGRP = 4
QB = NBLK // NGRP            # 25 blocks per quarter
QLOC = QB * 128              # 3200 rows per quarter
GRP = NCORES * QLOC          # 25600 rows per slab (int16-safe)
SB = 2                       # blocks per superblock (must divide NBLK)
NSUP = NBLK // SB
NSWQ = 4                     # SWDGE queues (desc-gen Q7 core pairs)


def _preprocess(edge_index, batch):
    # self-loops are NOT streamed as edges; they are applied on-chip via an
    # identity-seeded PSUM matmul. deg still counts them (reference adds
    # loops before computing symmetric normalization).
    src_g = np.asarray(edge_index[0], np.int64)
    dst_g = np.asarray(edge_index[1], np.int64)

    deg = (np.bincount(dst_g, minlength=N) + 1).astype(np.float64)
    dis = (1.0 / np.sqrt(np.maximum(deg, 1.0))).astype(np.float32)

    # snake-balanced node -> packed-slot assignment: deal nodes (sorted by
    # in-degree desc) across all NCORES*NBLK blocks so per-(core,blk,grp)
    # edge counts equalize -> smaller chunk-count C (less Q7 desc-gen).
    nblk_all = NCORES * NBLK
    order = np.argsort(-deg, kind="stable")
    pos = np.arange(N)
    cyc, r = pos // nblk_all, pos % nblk_all
    blk_of = np.where(cyc % 2 == 0, r, nblk_all - 1 - r)
    rank_of = cyc
    perm = np.empty(N, dtype=np.int64)
    gblk = blk_of
    perm[order] = (gblk // NBLK) * NLOC + (gblk % NBLK) * 128 + rank_of
    assert rank_of.max() < 128

    src_p = perm[src_g]
    dst_p = perm[dst_g]

    core = dst_p // NLOC
    blk = (dst_p % NLOC) // 128
    dcol = dst_p % 128
    # source slab: quarter q of the owning core; slab-local index
    src_core = src_p // NLOC
    src_loc = src_p % NLOC
    grp = src_loc // QLOC
    lidx = src_core * QLOC + src_loc % QLOC

    order = np.lexsort((dcol, blk, grp, blk // SB, core))
    core_s, blk_s, grp_s, dcol_s, lidx_s = (
        core[order], blk[order], grp[order], dcol[order], lidx[order])

    sup_s = blk_s // SB
    # per (core, sup, g) counts; A = first block of sup
    key_sg = (core_s * NSUP + sup_s) * NGRP + grp_s
    cnt_sg = np.bincount(key_sg, minlength=NCORES * NSUP * NGRP)
    cnt_sg = cnt_sg.reshape(NCORES, NSUP, NGRP)
    isA = (blk_s % SB) == 0
    cnt_a = np.bincount(key_sg[isA], minlength=NCORES * NSUP * NGRP)
    cnt_a = cnt_a.reshape(NCORES, NSUP, NGRP)

    n_sg = np.maximum(cnt_sg.max(axis=0), 1)       # (NSUP, NGRP)
    # uniform chunk count: every (s,g) bucket gets Cu chunks so all msg
    # tiles share one shape (fixed pool buffers -> trimmed gathers only
    # expose bytes already written by an earlier full gather)
    Cu = int(np.ceil(n_sg / 128).max())
    C = np.full((NSUP, NGRP), Cu, dtype=np.int64)
    cA = np.maximum(np.ceil(cnt_a / 128).astype(np.int64).max(axis=0), 1)
    cA = np.minimum(cA, C)
    cB0 = np.minimum((cnt_a // 128).min(axis=0), C - 1)
    Ctot = int(C.sum())
    S = Ctot * 128

    # slot offsets in stream order (sup, g, chunks)
    off_sg = np.zeros((NSUP, NGRP), dtype=np.int64)
    acc = 0
    for s in range(NSUP):
        for g in range(NGRP):
            off_sg[s, g] = acc
            acc += C[s, g] * 128
    assert acc == S

    # idx stream: real idxs, then dummy-0 up to n_sg (so every core's
    # trailing-negative trim stops at exactly n_sg == num_idxs_reg), then -1
    idx_stream = np.full((NCORES, S), -1, dtype=np.int16)
    dl_stream = np.full((NCORES, S), -1.0, dtype=np.float16)

    EA = len(key_sg)
    is_start = np.ones(EA, dtype=bool)
    is_start[1:] = key_sg[1:] != key_sg[:-1]
    run_start_idx = np.flatnonzero(is_start)
    run_id = np.cumsum(is_start) - 1
    run_pos = np.arange(EA) - run_start_idx[run_id]
    slot = off_sg[sup_s, grp_s] + run_pos
    idx_stream[core_s, slot] = lidx_s.astype(np.int16)
    dl_stream[core_s, slot] = ((blk_s % SB) * 128 + dcol_s).astype(np.float16)

    # superblocks 0,1 gather untrimmed (all idxs valid): they write every
    # byte of both msg pool buffers, so later trimmed gathers only expose
    # finite stale table data (never uninitialized Inf/NaN, which would
    # poison the Sel matmul via 0 * Inf)
    for k in range(NCORES):
        for s in range(NSUP):
            for g in range(NGRP):
                a = off_sg[s, g] + cnt_sg[k, s, g]
                b = off_sg[s, g] + (C[s, g] * 128 if s < 2 else n_sg[s, g])
                if b > a:
                    idx_stream[k, a:b] = 0

    return dis, (C, cA, cB0, n_sg), idx_stream, dl_stream, perm


def _build_nc(Cm):
    C, cA, cB0, n_sg = Cm
    NGRPL = NGRP
    Ctot = int(C.sum())
    S = Ctot * 128
    C_sup_g = C
    off_sup = np.zeros(NSUP + 1, dtype=np.int64)
    for s in range(NSUP):
        off_sup[s + 1] = off_sup[s] + C[s].sum()

    # multiple SWDGE queues: dma_gather desc-gen for queue q runs on Q7 core
    # pair (2q, 2q+1), so different queues' descriptor generation overlaps
    nc = bacc_mod.Bacc(num_swdge_queues=NSWQ)

    xT = nc.declare_dram_parameter("xT", [IN_C, NLOC], F16, isOutput=False)
    disN = nc.declare_dram_parameter("disN", [NLOC, 1], F32, isOutput=False)
    disT = nc.declare_dram_parameter("disT", [1, NLOC], F32, isOutput=False)
    W0 = nc.declare_dram_parameter("W0", [IN_C, HID], F16, isOutput=False)
    W1 = nc.declare_dram_parameter("W1", [HID, HID], F16, isOutput=False)
    W2 = nc.declare_dram_parameter("W2", [HID, HID], F16, isOutput=False)
    b0 = nc.declare_dram_parameter("b0", [HID, 1], F32, isOutput=False)
    b1 = nc.declare_dram_parameter("b1", [HID, 1], F32, isOutput=False)
    b2 = nc.declare_dram_parameter("b2", [HID, 1], F32, isOutput=False)
    Wm1 = nc.declare_dram_parameter("Wm1", [HID, HID // 2], F32, isOutput=False)
    bm1 = nc.declare_dram_parameter("bm1", [HID // 2, 1], F32, isOutput=False)
    Wm2 = nc.declare_dram_parameter("Wm2", [HID // 2, 1], F32, isOutput=False)
    bm2 = nc.declare_dram_parameter("bm2", [1, 1], F32, isOutput=False)
    idx16 = nc.declare_dram_parameter("idx16", [128, S // 16], I16, isOutput=False)
    dl16 = nc.declare_dram_parameter("dl16", [128, Ctot], F16, isOutput=False)
    segsel = nc.declare_dram_parameter("segsel", [NLOC, B], F16, isOutput=False)
    invcnt = nc.declare_dram_parameter("invcnt", [B, 1], F32, isOutput=False)
    iota128 = nc.declare_dram_parameter("iota128", [128, 256], F16, isOutput=False)
    ident64 = nc.declare_dram_parameter("ident64", [HID, HID], F16, isOutput=False)
    ident128 = nc.declare_dram_parameter("ident128", [128, 128], F16, isOutput=False)
    identB = nc.declare_dram_parameter("identB", [B, B], F32, isOutput=False)
    z = nc.declare_dram_parameter("z", [1, B], F32, isOutput=True)

    t_loc_q = [nc.dram_tensor(f"t_loc{q}", [QLOC, 128], F16)
               for q in range(NGRP)]
    t_full_q = [nc.dram_tensor(f"t_full{q}", [GRP, 128], F16,
                               addr_space="Shared")
                for q in range(NGRP)]
    pool_in = nc.dram_tensor("pool_in", [B, HID], F32)
    pool_out = nc.dram_tensor("pool_out", [B, HID], F32, addr_space="Shared")

    groups = [list(range(NCORES))]

    with tile.TileContext(nc) as tc:
        with (
            tc.tile_pool(name="const", bufs=1) as constp,
            tc.tile_pool(name="hT", bufs=1) as hTp,
            tc.tile_pool(name="tstage", bufs=1) as tstp,
            tc.tile_pool(name="xblk", bufs=3) as xblkp,
            tc.tile_pool(name="idx", bufs=2) as idxp,
            tc.tile_pool(name="msg", bufs=2) as msgp,
            tc.tile_pool(name="sel", bufs=2) as selp,
            tc.tile_pool(name="eps", bufs=3) as epsp,
            tc.tile_pool(name="psA", bufs=2, space="PSUM") as psA,
            tc.tile_pool(name="psB", bufs=2, space="PSUM") as psB,
            tc.tile_pool(name="psPool", bufs=1, space="PSUM") as psPoolp,
        ):
            disN_sb = constp.tile([128, NBLK], F32)
            nc.sync.dma_start(
                out=disN_sb[:], in_=disN.rearrange("(b p) o -> p (b o)", p=128))
            W0_sb = constp.tile([IN_C, HID], F16)
            nc.sync.dma_start(out=W0_sb[:], in_=W0[:])
            W1_sb = constp.tile([HID, HID], F16)
            nc.sync.dma_start(out=W1_sb[:], in_=W1[:])
            W2_sb = constp.tile([HID, HID], F16)
            nc.sync.dma_start(out=W2_sb[:], in_=W2[:])
            bias_sb = constp.tile([HID, 3], F32)
            for i, bb in enumerate([b0, b1, b2]):
                nc.sync.dma_start(out=bias_sb[:, i:i + 1], in_=bb[:])
            iota_sb = constp.tile([128, 256], F16)
            nc.sync.dma_start(out=iota_sb[:], in_=iota128[:])
            dl_sb = constp.tile([128, Ctot], F16)
            nc.sync.dma_start(out=dl_sb[:], in_=dl16[:])
            seg_sb = constp.tile([128, NBLK, B], F16)
            nc.sync.dma_start(
                out=seg_sb[:], in_=segsel.rearrange("(b p) g -> p b g", p=128))
            ident64_sb = constp.tile([HID, HID], F16)
            nc.sync.dma_start(out=ident64_sb[:], in_=ident64[:])
            ident128_sb = constp.tile([128, 128], F16)
            nc.sync.dma_start(out=ident128_sb[:], in_=ident128[:])
            identB_sb = constp.tile([B, B], F32)
            nc.sync.dma_start(out=identB_sb[:], in_=identB[:])
            invcnt_sb = constp.tile([B, 1], F32)
            nc.sync.dma_start(out=invcnt_sb[:], in_=invcnt[:])
            mlpw_sb = constp.tile([HID, HID // 2 + 1], F32)
            nc.sync.dma_start(out=mlpw_sb[:, :HID // 2], in_=Wm1[:])
            nc.sync.dma_start(out=mlpw_sb[:HID // 2, HID // 2:], in_=Wm2[:])
            bm_sb = constp.tile([HID // 2, 2], F32)
            nc.sync.dma_start(out=bm_sb[:, 0:1], in_=bm1[:])
            nc.sync.dma_start(out=bm_sb[0:1, 1:2], in_=bm2[:])

            hT = hTp.tile([HID, NLOC], F16, tag="hT")

            for l in range(3):
                tst = tstp.tile([128, NBLK, 128], F16, tag="tstage")
                for b in range(NBLK):
                    pt = psA.tile([128, HID], F32, tag="psA")
                    if l == 0:
                        xb = xblkp.tile([IN_C, 128], F16, tag="xblk")
                        nc.sync.dma_start(out=xb[:], in_=xT[:, b * 128:(b + 1) * 128])
                        nc.tensor.matmul(pt[:], xb[:], W0_sb[:], start=True, stop=True)
                    else:
                        W_sb = W1_sb if l == 1 else W2_sb
                        nc.tensor.matmul(
                            pt[:], hT[:, b * 128:(b + 1) * 128], W_sb[:],
                            start=True, stop=True)
                    nc.vector.tensor_tensor(
                        out=tst[:, b, 0:HID], in0=pt[:],
                        in1=disN_sb[:, b:b + 1].to_broadcast([128, HID]),
                        op=mybir.AluOpType.mult)
                    nc.vector.tensor_copy(out=tst[:, b, HID:128], in_=tst[:, b, 0:HID])
                # stage + AllGather per quarter: slab q lands on every core's
                # HBM while later slabs are still in flight; group-q gathers
                # only wait for their own slab
                for q in range(NGRP):
                    nc.sync.dma_start(
                        out=t_loc_q[q].rearrange("(b p) e -> p b e", p=128),
                        in_=tst[:, q * QB:(q + 1) * QB, :])
                    nc.gpsimd.collective_compute(
                        "AllGather", mybir.AluOpType.bypass,
                        replica_groups=groups,
                        ins=[t_loc_q[q][:]], outs=[t_full_q[q][:]])

                for s in range(NSUP):
                    sup_slot0 = int(off_sup[s]) * 128
                    sup_slots = int(C_sup_g[s].sum()) * 128
                    disTB = idxp.tile([HID, SB * 128], F32, tag="disTB")
                    nc.sync.dma_start(
                        out=disTB[:].unsqueeze(1),
                        in_=disT[:, s * SB * 128:(s + 1) * SB * 128]
                        .partition_broadcast(HID))
                    idxb = idxp.tile([128, sup_slots // 16], I16, tag="idx")
                    nc.sync.dma_start(
                        out=idxb[:],
                        in_=idx16[:, sup_slot0 // 16:(sup_slot0 + sup_slots) // 16])
                    mts, selAs, selBs = [], [], []
                    goff = 0
                    for g in range(NGRPL):
                        cg = int(C[s, g])
                        ca, cb0 = int(cA[s, g]), int(cB0[s, g])
                        dl0 = int(off_sup[s]) + goff
                        selA = selp.tile([128, ca, 128], F16, tag=f"selA{g}")
                        nc.vector.tensor_tensor(
                            out=selA[:],
                            in0=dl_sb[:, dl0:dl0 + ca]
                            .unsqueeze(2).to_broadcast([128, ca, 128]),
                            in1=iota_sb[:, 0:128].unsqueeze(1)
                            .to_broadcast([128, ca, 128]),
                            op=mybir.AluOpType.is_equal)
                        nb = cg - cb0
                        selB = selp.tile([128, nb, 128], F16, tag=f"selB{g}")
                        nc.vector.tensor_tensor(
                            out=selB[:],
                            in0=dl_sb[:, dl0 + cb0:dl0 + cg]
                            .unsqueeze(2).to_broadcast([128, nb, 128]),
                            in1=iota_sb[:, 128:256].unsqueeze(1)
                            .to_broadcast([128, nb, 128]),
                            op=mybir.AluOpType.is_equal)
                        selAs.append(selA)
                        selBs.append(selB)
                        mt = msgp.tile([128, cg, 128], F16, tag=f"msg{g}")
                        nidx = cg * 128
                        nc.gpsimd.dma_gather(
                            out_ap=mt[:],
                            in_ap=t_full_q[g][:, :],
                            idxs_ap=idxb[:, (goff * 128) // 16:
                                         (goff * 128 + nidx) // 16],
                            num_idxs=nidx,
                            num_idxs_reg=nidx if s < 2 else int(n_sg[s, g]),
                            elem_size=128,
                            single_packet=False, queue_num=g % NSWQ)
                        mts.append(mt)
                        goff += cg
                    for bi in range(SB):
                        b = s * SB + bi
                        agg = psB.tile([HID, 128], F32, tag="psB")
                        if bi == 0:
                            njob = int(cA[s].sum())
                        else:
                            njob = int((C[s] - cB0[s]).sum())
                        # seed with the self-loop term: agg = t_blk^T
                        # (epilogue's *dis makes it dis^2 * (hW), the
                        # reference's self-loop contribution)
                        nc.tensor.matmul(
                            agg[:], tst[:, b, 0:HID], ident128_sb[:],
                            start=True, stop=False)
                        j = 0
                        for g in range(NGRPL):
                            if bi == 0:
                                rng_c = range(int(cA[s, g]))
                                stile, soff = selAs[g], 0
                            else:
                                rng_c = range(int(cB0[s, g]), int(C[s, g]))
                                stile, soff = selBs[g], int(cB0[s, g])
                            for c in rng_c:
                                nc.tensor.matmul(
                                    agg[:], mts[g][:, c, 0:HID],
                                    stile[:, c - soff, :],
                                    start=False, stop=(j == njob - 1))
                                j += 1
                        ep = epsp.tile([HID, 128], F32, tag="eps")
                        nc.vector.tensor_tensor(
                            out=ep[:], in0=agg[:],
                            in1=disTB[:, bi * 128:(bi + 1) * 128],
                            op=mybir.AluOpType.mult)
                        nc.scalar.activation(
                            out=hT[:, b * 128:(b + 1) * 128], in_=ep[:],
                            func=mybir.ActivationFunctionType.Relu,
                            bias=bias_sb[:, l:l + 1])

            pool_ps = psPoolp.tile([B, HID], F32)
            for b in range(NBLK):
                h3t = psA.tile([128, HID], F32, tag="psA")
                nc.tensor.matmul(
                    h3t[:], hT[:, b * 128:(b + 1) * 128], ident64_sb[:],
                    start=True, stop=True)
                h3s = epsp.tile([128, HID], F16, tag="h3s")
                nc.vector.tensor_copy(out=h3s[:], in_=h3t[:])
                nc.tensor.matmul(
                    pool_ps[:], seg_sb[:, b, :], h3s[:],
                    start=(b == 0), stop=(b == NBLK - 1))
            pool_sb = epsp.tile([B, HID], F32, tag="poolsb")
            nc.vector.tensor_copy(out=pool_sb[:], in_=pool_ps[:])
            nc.sync.dma_start(out=pool_in[:], in_=pool_sb[:])
            nc.gpsimd.collective_compute(
                "AllReduce", mybir.AluOpType.add, replica_groups=groups,
                ins=[pool_in[:]], outs=[pool_out[:]])
            pooled = epsp.tile([B, HID], F32, tag="pooled")
            nc.sync.dma_start(out=pooled[:], in_=pool_out[:])
            pm = epsp.tile([B, HID], F32, tag="pm")
            nc.vector.tensor_tensor(
                out=pm[:], in0=pooled[:],
                in1=invcnt_sb[:].to_broadcast([B, HID]),
                op=mybir.AluOpType.mult)
            ppT = psA.tile([HID, B], F32, tag="psA")
            nc.tensor.matmul(ppT[:], pm[:], identB_sb[:], start=True, stop=True)
            pT = epsp.tile([HID, B], F32, tag="pT")
            nc.vector.tensor_copy(out=pT[:], in_=ppT[:])
            z1p = psB.tile([HID // 2, B], F32, tag="psB")
            nc.tensor.matmul(z1p[:], mlpw_sb[:, :HID // 2], pT[:], start=True, stop=True)
            z1 = epsp.tile([HID // 2, B], F32, tag="z1")
            nc.scalar.activation(
                out=z1[:], in_=z1p[:],
                func=mybir.ActivationFunctionType.Relu, bias=bm_sb[:, 0:1])
            z2p = psB.tile([1, B], F32, tag="psB")
            nc.tensor.matmul(
                z2p[:], mlpw_sb[:HID // 2, HID // 2:HID // 2 + 1], z1[:],
                start=True, stop=True)
            zf = epsp.tile([1, B], F32, tag="zf")
            nc.vector.tensor_tensor(
                out=zf[:], in0=z2p[:],
                in1=bm_sb[0:1, 1:2].to_broadcast([1, B]),
                op=mybir.AluOpType.add)
            nc.sync.dma_start(out=z[:], in_=zf[:])

    nc.finalize()
    return nc


_CACHE = {}


def kernel(x, edge_index, batch, W0, b0, W1, b1, W2, b2, Wm1, bm1, Wm2, bm2,
           trace=False):
    x = np.asarray(x, np.float32)
    dis, Cm, idx_stream, dl_stream, perm = _preprocess(
        np.asarray(edge_index), np.asarray(batch))
    C = Cm[0]
    Ctot = int(C.sum())
    S = Ctot * 128

    gid = np.asarray(batch, np.int64)
    cnts = np.bincount(gid, minlength=B).astype(np.float32)
    packed = perm

    xp = np.zeros((NCORES, IN_C, NLOC), np.float16)
    xp[packed // NLOC, :, packed % NLOC] = x.astype(np.float16)
    disp = np.zeros((NCORES, NLOC), np.float32)
    disp[packed // NLOC, packed % NLOC] = dis
    seg = np.zeros((NCORES, NLOC, B), np.float16)
    seg[packed // NLOC, packed % NLOC, gid] = 1.0

    iota = np.tile(np.arange(256, dtype=np.float16)[None, :], (128, 1))
    in_maps = []
    for k in range(NCORES):
        in_maps.append(dict(
            xT=xp[k],
            disN=disp[k][:, None].copy(),
            disT=disp[k][None, :].copy(),
            W0=np.asarray(W0, np.float16),
            W1=np.asarray(W1, np.float16),
            W2=np.asarray(W2, np.float16),
            b0=np.asarray(b0, np.float32)[:, None],
            b1=np.asarray(b1, np.float32)[:, None],
            b2=np.asarray(b2, np.float32)[:, None],
            Wm1=np.asarray(Wm1, np.float32),
            bm1=np.asarray(bm1, np.float32)[:, None],
            Wm2=np.asarray(Wm2, np.float32),
            bm2=np.asarray(bm2, np.float32).reshape(1, 1),
            idx16=np.tile(idx_stream[k].reshape(S // 16, 16).T, (8, 1)).copy(),
            dl16=dl_stream[k].reshape(Ctot, 128).T.copy(),
            segsel=seg[k],
            invcnt=(1.0 / np.maximum(cnts, 1.0)).astype(np.float32)[:, None],
            iota128=iota,
            ident64=np.eye(HID, dtype=np.float16),
            ident128=np.eye(128, dtype=np.float16),
            identB=np.eye(B, dtype=np.float32),
        ))

    ckey = C.tobytes() + Cm[1].tobytes() + Cm[2].tobytes() + Cm[3].tobytes()
    if ckey not in _CACHE:
        _CACHE[ckey] = _build_nc(Cm)
    nc = _CACHE[ckey]

    res = run_bass_kernel_spmd(nc, in_maps, list(range(NCORES)), trace=trace)
    out = res.results[0]["z"].reshape(B, 1).astype(np.float32)
    if trace:
        return out, res
    return out


# revision 21
# speedup vs baseline: 1.0655x; 1.0655x over previous
"""FootballGCN (3x GCNConv + mean-pool + MLP) on 8 TRN2 NeuronCores.

Self-contained: takes full inputs, shards internally, runs a Bass/Tile SPMD
kernel via run_bass_kernel_spmd, returns the full (B, 1) output.

Strategy (dst-sharded message passing, feature-major on-chip layout):
  - nodes packed into 8 cores x 12800 local slots (12500 real + pad)
  - per layer: table t = dis * (h @ W) built per 128-node block via one
    matmul (lhsT = feature-major h block -> node-major psum, no transpose),
    quantized f16, duplicated to 256B rows, staged per quarter and
    AllGathered slab-by-slab (4 pipelined collectives) to every core's HBM
  - edge pass: per superblock (SB dst blocks) x 4 source slabs (int16 index
    range; slab q is Q7-pair-q's SWDGE queue so desc-gen overlaps 4-way),
    dma_gather 256B rows with per-bucket trailing trim via num_idxs_reg;
    selection matrix built on DVE via is_equal(dst_local, iota); PE matmul
    (lhsT=msg, rhs=Sel) accumulates per-block agg[64, 128] in PSUM, seeded
    with the self-loop term t^T @ I; epilogue relu(dis*agg + b) on DVE+ACT
  - pooling via segsel matmuls, AllReduce, tiny MLP, output z
"""
import numpy as np

import concourse.bass as bass
import concourse.mybir as mybir
import concourse.tile as tile
from concourse import bacc as bacc_mod
from concourse.bass_utils import run_bass_kernel_spmd

F16 = mybir.dt.float16
F32 = mybir.dt.float32
I16 = mybir.dt.int16

# ---- problem dims (hardcoded per spec) ----
N = 100000
E = 3200000
B = 128
IN_C, HID = 128, 64
NCORES = 8
NREAL = 12500
NBLK = 100                   # blocks per core (multiple of 4 for slab split)
NLOC = NBLK * 128            # 12800
NTOT = NCORES * NLOC         # 102400
NGRP = 4
QB = NBLK // NGRP            # 25 blocks per quarter
QLOC = QB * 128              # 3200 rows per quarter
GRP = NCORES * QLOC          # 25600 rows per slab (int16-safe)
SB = 2                       # blocks per superblock (must divide NBLK)
NSUP = NBLK // SB
NSWQ = 4                     # SWDGE queues (desc-gen Q7 core pairs)


def _preprocess(edge_index, batch):
    # self-loops are NOT streamed as edges; they are applied on-chip via an
    # identity-seeded PSUM matmul. deg still counts them (reference adds
    # loops before computing symmetric normalization).
    src_g = np.asarray(edge_index[0], np.int64)
    dst_g = np.asarray(edge_index[1], np.int64)

    deg = (np.bincount(dst_g, minlength=N) + 1).astype(np.float64)
    dis = (1.0 / np.sqrt(np.maximum(deg, 1.0))).astype(np.float32)

    # snake-balanced node -> packed-slot assignment: deal nodes (sorted by
    # in-degree desc) across all NCORES*NBLK blocks so per-(core,blk,grp)
    # edge counts equalize -> smaller chunk-count C (less Q7 desc-gen).
    nblk_all = NCORES * NBLK
    order = np.argsort(-deg, kind="stable")
    pos = np.arange(N)
    cyc, r = pos // nblk_all, pos % nblk_all
    blk_of = np.where(cyc % 2 == 0, r, nblk_all - 1 - r)
    rank_of = cyc
    perm = np.empty(N, dtype=np.int64)
    gblk = blk_of
    perm[order] = (gblk // NBLK) * NLOC + (gblk % NBLK) * 128 + rank_of
    assert rank_of.max() < 128

    src_p = perm[src_g]
    dst_p = perm[dst_g]

    core = dst_p // NLOC
    blk = (dst_p % NLOC) // 128
    dcol = dst_p % 128
    # source slab: quarter q of the owning core; slab-local index
    src_core = src_p // NLOC
    src_loc = src_p % NLOC
    grp = src_loc // QLOC
    lidx = src_core * QLOC + src_loc % QLOC

    order = np.lexsort((dcol, blk, grp, blk // SB, core))
    core_s, blk_s, grp_s, dcol_s, lidx_s = (
        core[order], blk[order], grp[order], dcol[order], lidx[order])

    sup_s = blk_s // SB
    # per (core, sup, g) counts; A = first block of sup
    key_sg = (core_s * NSUP + sup_s) * NGRP + grp_s
    cnt_sg = np.bincount(key_sg, minlength=NCORES * NSUP * NGRP)
    cnt_sg = cnt_sg.reshape(NCORES, NSUP, NGRP)
    isA = (blk_s % SB) == 0
    cnt_a = np.bincount(key_sg[isA], minlength=NCORES * NSUP * NGRP)
    cnt_a = cnt_a.reshape(NCORES, NSUP, NGRP)

    n_sg = np.maximum(cnt_sg.max(axis=0), 1)       # (NSUP, NGRP)
    C = np.ceil(n_sg / 128).astype(np.int64)
    cA = np.maximum(np.ceil(cnt_a / 128).astype(np.int64).max(axis=0), 1)
    cA = np.minimum(cA, C)
    cB0 = np.minimum((cnt_a // 128).min(axis=0), C - 1)
    Ctot = int(C.sum())
    S = Ctot * 128

    # slot offsets in stream order (sup, g, chunks)
    off_sg = np.zeros((NSUP, NGRP), dtype=np.int64)
    acc = 0
    for s in range(NSUP):
        for g in range(NGRP):
            off_sg[s, g] = acc
            acc += C[s, g] * 128
    assert acc == S

    # idx stream: real idxs, then dummy-0 up to n_sg (so every core's
    # trailing-negative trim stops at exactly n_sg == num_idxs_reg), then -1
    idx_stream = np.full((NCORES, S), -1, dtype=np.int16)
    dl_stream = np.full((NCORES, S), -1.0, dtype=np.float16)

    EA = len(key_sg)
    is_start = np.ones(EA, dtype=bool)
    is_start[1:] = key_sg[1:] != key_sg[:-1]
    run_start_idx = np.flatnonzero(is_start)
    run_id = np.cumsum(is_start) - 1
    run_pos = np.arange(EA) - run_start_idx[run_id]
    slot = off_sg[sup_s, grp_s] + run_pos
    idx_stream[core_s, slot] = lidx_s.astype(np.int16)
    dl_stream[core_s, slot] = ((blk_s % SB) * 128 + dcol_s).astype(np.float16)

    # all pad slots gather row 0 (valid, dl=-1 keeps Sel at 0) so every mt
    # lane is written — a skipped lane would leave stale SBUF that can be
    # Inf/NaN, and 0 * Inf = NaN through the Sel matmul
    idx_stream[idx_stream < 0] = 0

    return dis, (C, cA, cB0, n_sg), idx_stream, dl_stream, perm


def _build_nc(Cm):
    C, cA, cB0, n_sg = Cm
    NGRPL = NGRP
    Ctot = int(C.sum())
    S = Ctot * 128
    C_sup_g = C
    off_sup = np.zeros(NSUP + 1, dtype=np.int64)
    for s in range(NSUP):
        off_sup[s + 1] = off_sup[s] + C[s].sum()

    # multiple SWDGE queues: dma_gather desc-gen for queue q runs on Q7 core
    # pair (2q, 2q+1), so different queues' descriptor generation overlaps
    nc = bacc_mod.Bacc(num_swdge_queues=NSWQ)

    xT = nc.declare_dram_parameter("xT", [IN_C, NLOC], F16, isOutput=False)
    disN = nc.declare_dram_parameter("disN", [NLOC, 1], F32, isOutput=False)
    disT = nc.declare_dram_parameter("disT", [1, NLOC], F32, isOutput=False)
    W0 = nc.declare_dram_parameter("W0", [IN_C, HID], F16, isOutput=False)
    W1 = nc.declare_dram_parameter("W1", [HID, HID], F16, isOutput=False)
    W2 = nc.declare_dram_parameter("W2", [HID, HID], F16, isOutput=False)
    b0 = nc.declare_dram_parameter("b0", [HID, 1], F32, isOutput=False)
    b1 = nc.declare_dram_parameter("b1", [HID, 1], F32, isOutput=False)
    b2 = nc.declare_dram_parameter("b2", [HID, 1], F32, isOutput=False)
    Wm1 = nc.declare_dram_parameter("Wm1", [HID, HID // 2], F32, isOutput=False)
    bm1 = nc.declare_dram_parameter("bm1", [HID // 2, 1], F32, isOutput=False)
    Wm2 = nc.declare_dram_parameter("Wm2", [HID // 2, 1], F32, isOutput=False)
    bm2 = nc.declare_dram_parameter("bm2", [1, 1], F32, isOutput=False)
    idx16 = nc.declare_dram_parameter("idx16", [128, S // 16], I16, isOutput=False)
    dl16 = nc.declare_dram_parameter("dl16", [128, Ctot], F16, isOutput=False)
    segsel = nc.declare_dram_parameter("segsel", [NLOC, B], F16, isOutput=False)
    invcnt = nc.declare_dram_parameter("invcnt", [B, 1], F32, isOutput=False)
    iota128 = nc.declare_dram_parameter("iota128", [128, 256], F16, isOutput=False)
    ident64 = nc.declare_dram_parameter("ident64", [HID, HID], F16, isOutput=False)
    ident128 = nc.declare_dram_parameter("ident128", [128, 128], F16, isOutput=False)
    identB = nc.declare_dram_parameter("identB", [B, B], F32, isOutput=False)
    z = nc.declare_dram_parameter("z", [1, B], F32, isOutput=True)

    t_loc_q = [nc.dram_tensor(f"t_loc{q}", [QLOC, 128], F16)
               for q in range(NGRP)]
    t_full_q = [nc.dram_tensor(f"t_full{q}", [GRP, 128], F16,
                               addr_space="Shared")
                for q in range(NGRP)]
    pool_in = nc.dram_tensor("pool_in", [B, HID], F32)
    pool_out = nc.dram_tensor("pool_out", [B, HID], F32, addr_space="Shared")

    groups = [list(range(NCORES))]

    with tile.TileContext(nc) as tc:
        with (
            tc.tile_pool(name="const", bufs=1) as constp,
            tc.tile_pool(name="hT", bufs=1) as hTp,
            tc.tile_pool(name="tstage", bufs=1) as tstp,
            tc.tile_pool(name="xblk", bufs=2) as xblkp,
            tc.tile_pool(name="idx", bufs=2) as idxp,
            tc.tile_pool(name="msg", bufs=2) as msgp,
            tc.tile_pool(name="sel", bufs=2) as selp,
            tc.tile_pool(name="eps", bufs=3) as epsp,
            tc.tile_pool(name="psA", bufs=2, space="PSUM") as psA,
            tc.tile_pool(name="psB", bufs=2, space="PSUM") as psB,
            tc.tile_pool(name="psPool", bufs=1, space="PSUM") as psPoolp,
        ):
            disN_sb = constp.tile([128, NBLK], F32)
            nc.sync.dma_start(
                out=disN_sb[:], in_=disN.rearrange("(b p) o -> p (b o)", p=128))
            W0_sb = constp.tile([IN_C, HID], F16)
            nc.sync.dma_start(out=W0_sb[:], in_=W0[:])
            W1_sb = constp.tile([HID, HID], F16)
            nc.sync.dma_start(out=W1_sb[:], in_=W1[:])
            W2_sb = constp.tile([HID, HID], F16)
            nc.sync.dma_start(out=W2_sb[:], in_=W2[:])
            bias_sb = constp.tile([HID, 3], F32)
            for i, bb in enumerate([b0, b1, b2]):
                nc.sync.dma_start(out=bias_sb[:, i:i + 1], in_=bb[:])
            iota_sb = constp.tile([128, 256], F16)
            nc.sync.dma_start(out=iota_sb[:], in_=iota128[:])
            dl_sb = constp.tile([128, Ctot], F16)
            nc.sync.dma_start(out=dl_sb[:], in_=dl16[:])
            seg_sb = constp.tile([128, NBLK, B], F16)
            nc.sync.dma_start(
                out=seg_sb[:], in_=segsel.rearrange("(b p) g -> p b g", p=128))
            ident64_sb = constp.tile([HID, HID], F16)
            nc.sync.dma_start(out=ident64_sb[:], in_=ident64[:])
            ident128_sb = constp.tile([128, 128], F16)
            nc.sync.dma_start(out=ident128_sb[:], in_=ident128[:])
            identB_sb = constp.tile([B, B], F32)
            nc.sync.dma_start(out=identB_sb[:], in_=identB[:])
            invcnt_sb = constp.tile([B, 1], F32)
            nc.sync.dma_start(out=invcnt_sb[:], in_=invcnt[:])
            mlpw_sb = constp.tile([HID, HID // 2 + 1], F32)
            nc.sync.dma_start(out=mlpw_sb[:, :HID // 2], in_=Wm1[:])
            nc.sync.dma_start(out=mlpw_sb[:HID // 2, HID // 2:], in_=Wm2[:])
            bm_sb = constp.tile([HID // 2, 2], F32)
            nc.sync.dma_start(out=bm_sb[:, 0:1], in_=bm1[:])
            nc.sync.dma_start(out=bm_sb[0:1, 1:2], in_=bm2[:])

            hT = hTp.tile([HID, NLOC], F16, tag="hT")

            pool_ps = psPoolp.tile([B, HID], F32)

            for l in range(3):
                tst = tstp.tile([128, NBLK, 128], F16, tag="tstage")
                xq = None
                for b in range(NBLK):
                    pt = psA.tile([128, HID], F32, tag="psA")
                    if l == 0:
                        if b % QB == 0:
                            q0 = b // QB
                            xq = xblkp.tile([IN_C, QLOC], F16, tag="xq")
                            nc.sync.dma_start(
                                out=xq[:],
                                in_=xT[:, q0 * QLOC:(q0 + 1) * QLOC])
                        xb = xq[:, (b % QB) * 128:(b % QB + 1) * 128]
                        nc.tensor.matmul(pt[:], xb, W0_sb[:], start=True, stop=True)
                    else:
                        W_sb = W1_sb if l == 1 else W2_sb
                        nc.tensor.matmul(
                            pt[:], hT[:, b * 128:(b + 1) * 128], W_sb[:],
                            start=True, stop=True)
                    nc.vector.tensor_tensor(
                        out=tst[:, b, 0:HID], in0=pt[:],
                        in1=disN_sb[:, b:b + 1].to_broadcast([128, HID]),
                        op=mybir.AluOpType.mult)
                    nc.vector.tensor_copy(out=tst[:, b, HID:128], in_=tst[:, b, 0:HID])
                # stage + AllGather per quarter: slab q lands on every core's
                # HBM while later slabs are still in flight; group-q gathers
                # only wait for their own slab
                for q in range(NGRP):
                    nc.sync.dma_start(
                        out=t_loc_q[q].rearrange("(b p) e -> p b e", p=128),
                        in_=tst[:, q * QB:(q + 1) * QB, :])
                    nc.gpsimd.collective_compute(
                        "AllGather", mybir.AluOpType.bypass,
                        replica_groups=groups,
                        ins=[t_loc_q[q][:]], outs=[t_full_q[q][:]])

                for s in range(NSUP):
                    sup_slot0 = int(off_sup[s]) * 128
                    sup_slots = int(C_sup_g[s].sum()) * 128
                    disTB = idxp.tile([HID, SB * 128], F32, tag="disTB")
                    nc.sync.dma_start(
                        out=disTB[:].unsqueeze(1),
                        in_=disT[:, s * SB * 128:(s + 1) * SB * 128]
                        .partition_broadcast(HID))
                    idxb = idxp.tile([128, sup_slots // 16], I16, tag="idx")
                    nc.sync.dma_start(
                        out=idxb[:],
                        in_=idx16[:, sup_slot0 // 16:(sup_slot0 + sup_slots) // 16])
                    mts, selAs, selBs = [], [], []
                    goff = 0
                    for g in range(NGRPL):
                        cg = int(C[s, g])
                        ca, cb0 = int(cA[s, g]), int(cB0[s, g])
                        dl0 = int(off_sup[s]) + goff
                        selA = selp.tile([128, ca, 128], F16, tag=f"selA{g}")
                        nc.vector.tensor_tensor(
                            out=selA[:],
                            in0=dl_sb[:, dl0:dl0 + ca]
                            .unsqueeze(2).to_broadcast([128, ca, 128]),
                            in1=iota_sb[:, 0:128].unsqueeze(1)
                            .to_broadcast([128, ca, 128]),
                            op=mybir.AluOpType.is_equal)
                        nb = cg - cb0
                        selB = selp.tile([128, nb, 128], F16, tag=f"selB{g}")
                        nc.vector.tensor_tensor(
                            out=selB[:],
                            in0=dl_sb[:, dl0 + cb0:dl0 + cg]
                            .unsqueeze(2).to_broadcast([128, nb, 128]),
                            in1=iota_sb[:, 128:256].unsqueeze(1)
                            .to_broadcast([128, nb, 128]),
                            op=mybir.AluOpType.is_equal)
                        selAs.append(selA)
                        selBs.append(selB)
                        mt = msgp.tile([128, cg, 128], F16, tag=f"msg{g}")
                        nidx = cg * 128
                        nc.gpsimd.dma_gather(
                            out_ap=mt[:],
                            in_ap=t_full_q[g][:, :],
                            idxs_ap=idxb[:, (goff * 128) // 16:
                                         (goff * 128 + nidx) // 16],
                            num_idxs=nidx, num_idxs_reg=nidx,
                            elem_size=128,
                            single_packet=False, queue_num=g % NSWQ)
                        mts.append(mt)
                        goff += cg
                    for bi in range(SB):
                        b = s * SB + bi
                        agg = psB.tile([HID, 128], F32, tag="psB")
                        if bi == 0:
                            njob = int(cA[s].sum())
                        else:
                            njob = int((C[s] - cB0[s]).sum())
                        # seed with the self-loop term: agg = t_blk^T
                        # (epilogue's *dis makes it dis^2 * (hW), the
                        # reference's self-loop contribution)
                        nc.tensor.matmul(
                            agg[:], tst[:, b, 0:HID], ident128_sb[:],
                            start=True, stop=False)
                        j = 0
                        for g in range(NGRPL):
                            if bi == 0:
                                rng_c = range(int(cA[s, g]))
                                stile, soff = selAs[g], 0
                            else:
                                rng_c = range(int(cB0[s, g]), int(C[s, g]))
                                stile, soff = selBs[g], int(cB0[s, g])
                            for c in rng_c:
                                nc.tensor.matmul(
                                    agg[:], mts[g][:, c, 0:HID],
                                    stile[:, c - soff, :],
                                    start=False, stop=(j == njob - 1))
                                j += 1
                        ep = epsp.tile([HID, 128], F32, tag="eps")
                        nc.vector.tensor_tensor(
                            out=ep[:], in0=agg[:],
                            in1=disTB[:, bi * 128:(bi + 1) * 128],
                            op=mybir.AluOpType.mult)
                        nc.scalar.activation(
                            out=hT[:, b * 128:(b + 1) * 128], in_=ep[:],
                            func=mybir.ActivationFunctionType.Relu,
                            bias=bias_sb[:, l:l + 1])
                        if l == 2:
                            # pooling interleaved into layer 2's edge pass:
                            # PE is in-order, so emit per block as hT lands
                            h3t = psA.tile([128, HID], F32, tag="psA")
                            nc.tensor.matmul(
                                h3t[:], hT[:, b * 128:(b + 1) * 128],
                                ident64_sb[:], start=True, stop=True)
                            h3s = epsp.tile([128, HID], F16, tag="h3s")
                            nc.vector.tensor_copy(out=h3s[:], in_=h3t[:])
                            nc.tensor.matmul(
                                pool_ps[:], seg_sb[:, b, :], h3s[:],
                                start=(b == 0), stop=(b == NBLK - 1))

            pool_sb = epsp.tile([B, HID], F32, tag="poolsb")
            nc.vector.tensor_copy(out=pool_sb[:], in_=pool_ps[:])
            nc.sync.dma_start(out=pool_in[:], in_=pool_sb[:])
            nc.gpsimd.collective_compute(
                "AllReduce", mybir.AluOpType.add, replica_groups=groups,
                ins=[pool_in[:]], outs=[pool_out[:]])
            pooled = epsp.tile([B, HID], F32, tag="pooled")
            nc.sync.dma_start(out=pooled[:], in_=pool_out[:])
            pm = epsp.tile([B, HID], F32, tag="pm")
            nc.vector.tensor_tensor(
                out=pm[:], in0=pooled[:],
                in1=invcnt_sb[:].to_broadcast([B, HID]),
                op=mybir.AluOpType.mult)
            ppT = psA.tile([HID, B], F32, tag="psA")
            nc.tensor.matmul(ppT[:], pm[:], identB_sb[:], start=True, stop=True)
            pT = epsp.tile([HID, B], F32, tag="pT")
            nc.vector.tensor_copy(out=pT[:], in_=ppT[:])
            z1p = psB.tile([HID // 2, B], F32, tag="psB")
            nc.tensor.matmul(z1p[:], mlpw_sb[:, :HID // 2], pT[:], start=True, stop=True)
            z1 = epsp.tile([HID // 2, B], F32, tag="z1")
            nc.scalar.activation(
                out=z1[:], in_=z1p[:],
                func=mybir.ActivationFunctionType.Relu, bias=bm_sb[:, 0:1])
            z2p = psB.tile([1, B], F32, tag="psB")
            nc.tensor.matmul(
                z2p[:], mlpw_sb[:HID // 2, HID // 2:HID // 2 + 1], z1[:],
                start=True, stop=True)
            zf = epsp.tile([1, B], F32, tag="zf")
            nc.vector.tensor_tensor(
                out=zf[:], in0=z2p[:],
                in1=bm_sb[0:1, 1:2].to_broadcast([1, B]),
                op=mybir.AluOpType.add)
            nc.sync.dma_start(out=z[:], in_=zf[:])

    nc.finalize()
    return nc


_CACHE = {}


def kernel(x, edge_index, batch, W0, b0, W1, b1, W2, b2, Wm1, bm1, Wm2, bm2,
           trace=False):
    x = np.asarray(x, np.float32)
    dis, Cm, idx_stream, dl_stream, perm = _preprocess(
        np.asarray(edge_index), np.asarray(batch))
    C = Cm[0]
    Ctot = int(C.sum())
    S = Ctot * 128

    gid = np.asarray(batch, np.int64)
    cnts = np.bincount(gid, minlength=B).astype(np.float32)
    packed = perm

    xp = np.zeros((NCORES, IN_C, NLOC), np.float16)
    xp[packed // NLOC, :, packed % NLOC] = x.astype(np.float16)
    disp = np.zeros((NCORES, NLOC), np.float32)
    disp[packed // NLOC, packed % NLOC] = dis
    seg = np.zeros((NCORES, NLOC, B), np.float16)
    seg[packed // NLOC, packed % NLOC, gid] = 1.0

    iota = np.tile(np.arange(256, dtype=np.float16)[None, :], (128, 1))
    in_maps = []
    for k in range(NCORES):
        in_maps.append(dict(
            xT=xp[k],
            disN=disp[k][:, None].copy(),
            disT=disp[k][None, :].copy(),
            W0=np.asarray(W0, np.float16),
            W1=np.asarray(W1, np.float16),
            W2=np.asarray(W2, np.float16),
            b0=np.asarray(b0, np.float32)[:, None],
            b1=np.asarray(b1, np.float32)[:, None],
            b2=np.asarray(b2, np.float32)[:, None],
            Wm1=np.asarray(Wm1, np.float32),
            bm1=np.asarray(bm1, np.float32)[:, None],
            Wm2=np.asarray(Wm2, np.float32),
            bm2=np.asarray(bm2, np.float32).reshape(1, 1),
            idx16=np.tile(idx_stream[k].reshape(S // 16, 16).T, (8, 1)).copy(),
            dl16=dl_stream[k].reshape(Ctot, 128).T.copy(),
            segsel=seg[k],
            invcnt=(1.0 / np.maximum(cnts, 1.0)).astype(np.float32)[:, None],
            iota128=iota,
            ident64=np.eye(HID, dtype=np.float16),
            ident128=np.eye(128, dtype=np.float16),
            identB=np.eye(B, dtype=np.float32),
        ))

    ckey = C.tobytes() + Cm[1].tobytes() + Cm[2].tobytes() + Cm[3].tobytes()
    if ckey not in _CACHE:
        _CACHE[ckey] = _build_nc(Cm)
    nc = _CACHE[ckey]

    res = run_bass_kernel_spmd(nc, in_maps, list(range(NCORES)), trace=trace)
    out = res.results[0]["z"].reshape(B, 1).astype(np.float32)
    if trace:
        return out, res
    return out


# revision 23
# speedup vs baseline: 1.1914x; 1.1181x over previous
"""FootballGCN (3x GCNConv + mean-pool + MLP) on 8 TRN2 NeuronCores.

Self-contained: takes full inputs, shards internally, runs a Bass/Tile SPMD
kernel via run_bass_kernel_spmd, returns the full (B, 1) output.

Strategy (dst-sharded message passing, feature-major on-chip layout):
  - nodes packed into 8 cores x 12800 local slots (12500 real + pad)
  - table t = dis * (h @ W) built per 128-node block; layer l+1's table is
    computed incrementally DURING layer l's edge pass (hT blocks land
    progressively), and each finished quarter is immediately staged +
    AllGathered into the other t_full parity, hiding table+collective
    behind the previous layer's edge pass
  - edge pass: per superblock (SB dst blocks) x 4 source slabs (int16 index
    range; slab g is Q7-pair-g's SWDGE queue so desc-gen overlaps 4-way),
    dma_gather 256B rows; selection matrix built on DVE via
    is_equal(dst_local, iota); PE matmul (lhsT=msg, rhs=Sel) accumulates
    per-block agg[64, 128] in PSUM, seeded with the self-loop term t^T @ I;
    epilogue relu(dis*agg + b) on DVE+ACT
  - pooling interleaved into layer 2's edge pass; AllReduce; tiny MLP
"""
import numpy as np

import concourse.bass as bass
import concourse.mybir as mybir
import concourse.tile as tile
from concourse import bacc as bacc_mod
from concourse.bass_utils import run_bass_kernel_spmd

F16 = mybir.dt.float16
F32 = mybir.dt.float32
I16 = mybir.dt.int16

# ---- problem dims (hardcoded per spec) ----
N = 100000
E = 3200000
B = 128
IN_C, HID = 128, 64
NCORES = 8
NREAL = 12500
NBLK = 100                   # blocks per core (multiple of 4 for slab split)
NLOC = NBLK * 128            # 12800
NTOT = NCORES * NLOC         # 102400
NGRP = 4
QB = NBLK // NGRP            # 25 blocks per quarter
QLOC = QB * 128              # 3200 rows per quarter
GRP = NCORES * QLOC          # 25600 rows per slab (int16-safe)
SB = 2                       # blocks per superblock (must divide NBLK)
NSUP = NBLK // SB
NSWQ = 4                     # SWDGE queues (desc-gen Q7 core pairs)


def _preprocess(edge_index, batch):
    # self-loops are NOT streamed as edges; they are applied on-chip via an
    # identity-seeded PSUM matmul. deg still counts them (reference adds
    # loops before computing symmetric normalization).
    src_g = np.asarray(edge_index[0], np.int64)
    dst_g = np.asarray(edge_index[1], np.int64)

    deg = (np.bincount(dst_g, minlength=N) + 1).astype(np.float64)
    dis = (1.0 / np.sqrt(np.maximum(deg, 1.0))).astype(np.float32)

    # snake-balanced node -> packed-slot assignment: deal nodes (sorted by
    # in-degree desc) across all NCORES*NBLK blocks so per-(core,blk,grp)
    # edge counts equalize -> smaller chunk-count C (less Q7 desc-gen).
    nblk_all = NCORES * NBLK
    order = np.argsort(-deg, kind="stable")
    pos = np.arange(N)
    cyc, r = pos // nblk_all, pos % nblk_all
    blk_of = np.where(cyc % 2 == 0, r, nblk_all - 1 - r)
    rank_of = cyc
    perm = np.empty(N, dtype=np.int64)
    gblk = blk_of
    perm[order] = (gblk // NBLK) * NLOC + (gblk % NBLK) * 128 + rank_of
    assert rank_of.max() < 128

    src_p = perm[src_g]
    dst_p = perm[dst_g]

    core = dst_p // NLOC
    blk = (dst_p % NLOC) // 128
    dcol = dst_p % 128
    # source slab: quarter q of the owning core; slab-local index
    src_core = src_p // NLOC
    src_loc = src_p % NLOC
    grp = src_loc // QLOC
    lidx = src_core * QLOC + src_loc % QLOC

    order = np.lexsort((dcol, blk, grp, blk // SB, core))
    core_s, blk_s, grp_s, dcol_s, lidx_s = (
        core[order], blk[order], grp[order], dcol[order], lidx[order])

    sup_s = blk_s // SB
    # per (core, sup, g) counts; A = first block of sup
    key_sg = (core_s * NSUP + sup_s) * NGRP + grp_s
    cnt_sg = np.bincount(key_sg, minlength=NCORES * NSUP * NGRP)
    cnt_sg = cnt_sg.reshape(NCORES, NSUP, NGRP)
    isA = (blk_s % SB) == 0
    cnt_a = np.bincount(key_sg[isA], minlength=NCORES * NSUP * NGRP)
    cnt_a = cnt_a.reshape(NCORES, NSUP, NGRP)

    n_sg = np.maximum(cnt_sg.max(axis=0), 1)       # (NSUP, NGRP)
    C = np.ceil(n_sg / 128).astype(np.int64)
    cA = np.maximum(np.ceil(cnt_a / 128).astype(np.int64).max(axis=0), 1)
    cA = np.minimum(cA, C)
    cB0 = np.minimum((cnt_a // 128).min(axis=0), C - 1)
    Ctot = int(C.sum())
    S = Ctot * 128

    # slot offsets in stream order (sup, g, chunks)
    off_sg = np.zeros((NSUP, NGRP), dtype=np.int64)
    acc = 0
    for s in range(NSUP):
        for g in range(NGRP):
            off_sg[s, g] = acc
            acc += C[s, g] * 128
    assert acc == S

    idx_stream = np.full((NCORES, S), -1, dtype=np.int16)
    dl_stream = np.full((NCORES, S), -1.0, dtype=np.float16)

    EA = len(key_sg)
    is_start = np.ones(EA, dtype=bool)
    is_start[1:] = key_sg[1:] != key_sg[:-1]
    run_start_idx = np.flatnonzero(is_start)
    run_id = np.cumsum(is_start) - 1
    run_pos = np.arange(EA) - run_start_idx[run_id]
    slot = off_sg[sup_s, grp_s] + run_pos
    idx_stream[core_s, slot] = lidx_s.astype(np.int16)
    dl_stream[core_s, slot] = ((blk_s % SB) * 128 + dcol_s).astype(np.float16)

    # all pad slots gather row 0 (valid, dl=-1 keeps Sel at 0) so every mt
    # lane is written — a skipped lane would leave stale SBUF that can be
    # Inf/NaN, and 0 * Inf = NaN through the Sel matmul
    idx_stream[idx_stream < 0] = 0

    return dis, (C, cA, cB0, n_sg), idx_stream, dl_stream, perm


def _build_nc(Cm):
    C, cA, cB0, n_sg = Cm
    NGRPL = NGRP
    Ctot = int(C.sum())
    S = Ctot * 128
    C_sup_g = C
    off_sup = np.zeros(NSUP + 1, dtype=np.int64)
    for s in range(NSUP):
        off_sup[s + 1] = off_sup[s] + C[s].sum()

    # multiple SWDGE queues: dma_gather desc-gen for queue q runs on Q7 core
    # pair (2q, 2q+1), so different queues' descriptor generation overlaps
    nc = bacc_mod.Bacc(num_swdge_queues=NSWQ)

    xT = nc.declare_dram_parameter("xT", [IN_C, NLOC], F16, isOutput=False)
    disN = nc.declare_dram_parameter("disN", [NLOC, 1], F32, isOutput=False)
    disT = nc.declare_dram_parameter("disT", [1, NLOC], F32, isOutput=False)
    W0 = nc.declare_dram_parameter("W0", [IN_C, HID], F16, isOutput=False)
    W1 = nc.declare_dram_parameter("W1", [HID, HID], F16, isOutput=False)
    W2 = nc.declare_dram_parameter("W2", [HID, HID], F16, isOutput=False)
    b0 = nc.declare_dram_parameter("b0", [HID, 1], F32, isOutput=False)
    b1 = nc.declare_dram_parameter("b1", [HID, 1], F32, isOutput=False)
    b2 = nc.declare_dram_parameter("b2", [HID, 1], F32, isOutput=False)
    Wm1 = nc.declare_dram_parameter("Wm1", [HID, HID // 2], F32, isOutput=False)
    bm1 = nc.declare_dram_parameter("bm1", [HID // 2, 1], F32, isOutput=False)
    Wm2 = nc.declare_dram_parameter("Wm2", [HID // 2, 1], F32, isOutput=False)
    bm2 = nc.declare_dram_parameter("bm2", [1, 1], F32, isOutput=False)
    idx16 = nc.declare_dram_parameter("idx16", [128, S // 16], I16, isOutput=False)
    dl16 = nc.declare_dram_parameter("dl16", [128, Ctot], F16, isOutput=False)
    segsel = nc.declare_dram_parameter("segsel", [NLOC, B], F16, isOutput=False)
    invcnt = nc.declare_dram_parameter("invcnt", [B, 1], F32, isOutput=False)
    iota128 = nc.declare_dram_parameter("iota128", [128, 256], F16, isOutput=False)
    ident64 = nc.declare_dram_parameter("ident64", [HID, HID], F16, isOutput=False)
    ident128 = nc.declare_dram_parameter("ident128", [128, 128], F16, isOutput=False)
    identB = nc.declare_dram_parameter("identB", [B, B], F32, isOutput=False)
    z = nc.declare_dram_parameter("z", [1, B], F32, isOutput=True)

    t_loc_q = [nc.dram_tensor(f"t_loc{q}", [QLOC, 128], F16)
               for q in range(NGRP)]
    # double-buffered by layer parity: layer l reads parity l%2 while layer
    # l+1's AllGathers (issued mid-edge-pass) write parity (l+1)%2
    t_full_q = [[nc.dram_tensor(f"t_full{p}_{q}", [GRP, 128], F16,
                                addr_space="Shared")
                 for q in range(NGRP)] for p in range(2)]
    pool_in = nc.dram_tensor("pool_in", [B, HID], F32)
    pool_out = nc.dram_tensor("pool_out", [B, HID], F32, addr_space="Shared")

    groups = [list(range(NCORES))]

    with tile.TileContext(nc) as tc:
        with (
            tc.tile_pool(name="const", bufs=1) as constp,
            tc.tile_pool(name="hT", bufs=1) as hTp,
            tc.tile_pool(name="tstage", bufs=2) as tstp,
            tc.tile_pool(name="xblk", bufs=2) as xblkp,
            tc.tile_pool(name="idx", bufs=2) as idxp,
            tc.tile_pool(name="msg", bufs=2) as msgp,
            tc.tile_pool(name="sel", bufs=2) as selp,
            tc.tile_pool(name="eps", bufs=3) as epsp,
            tc.tile_pool(name="psA", bufs=2, space="PSUM") as psA,
            tc.tile_pool(name="psB", bufs=2, space="PSUM") as psB,
            tc.tile_pool(name="psPool", bufs=1, space="PSUM") as psPoolp,
        ):
            disN_sb = constp.tile([128, NBLK], F32)
            nc.sync.dma_start(
                out=disN_sb[:], in_=disN.rearrange("(b p) o -> p (b o)", p=128))
            W0_sb = constp.tile([IN_C, HID], F16)
            nc.sync.dma_start(out=W0_sb[:], in_=W0[:])
            W1_sb = constp.tile([HID, HID], F16)
            nc.sync.dma_start(out=W1_sb[:], in_=W1[:])
            W2_sb = constp.tile([HID, HID], F16)
            nc.sync.dma_start(out=W2_sb[:], in_=W2[:])
            bias_sb = constp.tile([HID, 3], F32)
            for i, bb in enumerate([b0, b1, b2]):
                nc.sync.dma_start(out=bias_sb[:, i:i + 1], in_=bb[:])
            iota_sb = constp.tile([128, 256], F16)
            nc.sync.dma_start(out=iota_sb[:], in_=iota128[:])
            dl_sb = constp.tile([128, Ctot], F16)
            nc.sync.dma_start(out=dl_sb[:], in_=dl16[:])
            ident128_sb = constp.tile([128, 128], F16)
            nc.sync.dma_start(out=ident128_sb[:], in_=ident128[:])

            hT = hTp.tile([HID, NLOC], F16, tag="hT")
            pool_ps = psPoolp.tile([B, HID], F32)

            def table_mult(tstc, b, pt):
                nc.vector.tensor_tensor(
                    out=tstc[:, b, :], in0=pt[:],
                    in1=disN_sb[:, b:b + 1].to_broadcast([128, HID]),
                    op=mybir.AluOpType.mult)

            def stage_quarter(tstc, par, q):
                # duplicate the 64-wide table rows into 256B [t|t] rows via
                # two DMAs, then AllGather this slab into parity `par`
                tl = t_loc_q[q].rearrange("(b p) (h e) -> p b h e",
                                          p=128, h=2)
                src = tstc[:, q * QB:(q + 1) * QB, :].unsqueeze(2)
                for half in range(2):
                    nc.sync.dma_start(out=tl[:, :, half:half + 1, :], in_=src)
                nc.gpsimd.collective_compute(
                    "AllGather", mybir.AluOpType.bypass,
                    replica_groups=groups,
                    ins=[t_loc_q[q][:]], outs=[t_full_q[par][q][:]])

            # layer-0 table prologue (parity 0)
            tst0 = tstp.tile([128, NBLK, HID], F16, tag="tstage")
            tsts = [tst0]
            xq = None
            for b in range(NBLK):
                if b % QB == 0:
                    xq = xblkp.tile([IN_C, QLOC], F16, tag="xq")
                    nc.sync.dma_start(
                        out=xq[:],
                        in_=xT[:, (b // QB) * QLOC:(b // QB + 1) * QLOC])
                pt = psA.tile([128, HID], F32, tag="psA")
                nc.tensor.matmul(
                    pt[:], xq[:, (b % QB) * 128:(b % QB + 1) * 128],
                    W0_sb[:], start=True, stop=True)
                table_mult(tsts[0], b, pt)
                if (b + 1) % QB == 0:
                    stage_quarter(tsts[0], 0, b // QB)

            # late constants: not needed until layer 2 / the tail, so load
            # them behind the layer-0 edge pass instead of ahead of it
            seg_sb = constp.tile([128, NBLK, B], F16)
            nc.sync.dma_start(
                out=seg_sb[:], in_=segsel.rearrange("(b p) g -> p b g", p=128))
            ident64_sb = constp.tile([HID, HID], F16)
            nc.sync.dma_start(out=ident64_sb[:], in_=ident64[:])
            identB_sb = constp.tile([B, B], F32)
            nc.sync.dma_start(out=identB_sb[:], in_=identB[:])
            invcnt_sb = constp.tile([B, 1], F32)
            nc.sync.dma_start(out=invcnt_sb[:], in_=invcnt[:])
            mlpw_sb = constp.tile([HID, HID // 2 + 1], F32)
            nc.sync.dma_start(out=mlpw_sb[:, :HID // 2], in_=Wm1[:])
            nc.sync.dma_start(out=mlpw_sb[:HID // 2, HID // 2:], in_=Wm2[:])
            bm_sb = constp.tile([HID // 2, 2], F32)
            nc.sync.dma_start(out=bm_sb[:, 0:1], in_=bm1[:])
            nc.sync.dma_start(out=bm_sb[0:1, 1:2], in_=bm2[:])

            for l in range(3):
                par = l % 2
                tstc = tsts[l]
                if l < 2:
                    tstn = tstp.tile([128, NBLK, HID], F16, tag="tstage")
                    tsts.append(tstn)
                    Wn_sb = W1_sb if l == 0 else W2_sb

                for s in range(NSUP):
                    sup_slot0 = int(off_sup[s]) * 128
                    sup_slots = int(C_sup_g[s].sum()) * 128
                    disTB = idxp.tile([HID, SB * 128], F32, tag="disTB")
                    nc.sync.dma_start(
                        out=disTB[:].unsqueeze(1),
                        in_=disT[:, s * SB * 128:(s + 1) * SB * 128]
                        .partition_broadcast(HID))
                    idxb = idxp.tile([128, sup_slots // 16], I16, tag="idx")
                    nc.sync.dma_start(
                        out=idxb[:],
                        in_=idx16[:, sup_slot0 // 16:(sup_slot0 + sup_slots) // 16])
                    mts, selAs, selBs = [], [], []
                    goff = 0
                    for g in range(NGRPL):
                        cg = int(C[s, g])
                        ca, cb0 = int(cA[s, g]), int(cB0[s, g])
                        dl0 = int(off_sup[s]) + goff
                        selA = selp.tile([128, ca, 128], F16, tag=f"selA{g}")
                        nc.vector.tensor_tensor(
                            out=selA[:],
                            in0=dl_sb[:, dl0:dl0 + ca]
                            .unsqueeze(2).to_broadcast([128, ca, 128]),
                            in1=iota_sb[:, 0:128].unsqueeze(1)
                            .to_broadcast([128, ca, 128]),
                            op=mybir.AluOpType.is_equal)
                        nb = cg - cb0
                        selB = selp.tile([128, nb, 128], F16, tag=f"selB{g}")
                        nc.vector.tensor_tensor(
                            out=selB[:],
                            in0=dl_sb[:, dl0 + cb0:dl0 + cg]
                            .unsqueeze(2).to_broadcast([128, nb, 128]),
                            in1=iota_sb[:, 128:256].unsqueeze(1)
                            .to_broadcast([128, nb, 128]),
                            op=mybir.AluOpType.is_equal)
                        selAs.append(selA)
                        selBs.append(selB)
                        mt = msgp.tile([128, cg, 128], F16, tag=f"msg{g}")
                        nidx = cg * 128
                        nc.gpsimd.dma_gather(
                            out_ap=mt[:],
                            in_ap=t_full_q[par][g][:, :],
                            idxs_ap=idxb[:, (goff * 128) // 16:
                                         (goff * 128 + nidx) // 16],
                            num_idxs=nidx, num_idxs_reg=nidx,
                            elem_size=128,
                            single_packet=False, queue_num=g % NSWQ)
                        mts.append(mt)
                        goff += cg
                    for bi in range(SB):
                        b = s * SB + bi
                        agg = psB.tile([HID, 128], F32, tag="psB")
                        if bi == 0:
                            njob = int(cA[s].sum())
                        else:
                            njob = int((C[s] - cB0[s]).sum())
                        # seed with the self-loop term: agg = t_blk^T
                        # (epilogue's *dis makes it dis^2 * (hW), the
                        # reference's self-loop contribution)
                        nc.tensor.matmul(
                            agg[:], tstc[:, b, :], ident128_sb[:],
                            start=True, stop=False)
                        j = 0
                        for g in range(NGRPL):
                            if bi == 0:
                                rng_c = range(int(cA[s, g]))
                                stile, soff = selAs[g], 0
                            else:
                                rng_c = range(int(cB0[s, g]), int(C[s, g]))
                                stile, soff = selBs[g], int(cB0[s, g])
                            for c in rng_c:
                                nc.tensor.matmul(
                                    agg[:], mts[g][:, c, 0:HID],
                                    stile[:, c - soff, :],
                                    start=False, stop=(j == njob - 1))
                                j += 1
                        ep = epsp.tile([HID, 128], F32, tag="eps")
                        nc.vector.tensor_tensor(
                            out=ep[:], in0=agg[:],
                            in1=disTB[:, bi * 128:(bi + 1) * 128],
                            op=mybir.AluOpType.mult)
                        nc.scalar.activation(
                            out=hT[:, b * 128:(b + 1) * 128], in_=ep[:],
                            func=mybir.ActivationFunctionType.Relu,
                            bias=bias_sb[:, l:l + 1])
                        if l < 2:
                            # next layer's table block b, immediately after
                            # its hT lands; each finished quarter is staged
                            # and AllGathered into the other parity while
                            # this layer's edge pass continues
                            pt = psA.tile([128, HID], F32, tag="psA")
                            nc.tensor.matmul(
                                pt[:], hT[:, b * 128:(b + 1) * 128],
                                Wn_sb[:], start=True, stop=True)
                            table_mult(tstn, b, pt)
                            if (b + 1) % QB == 0:
                                stage_quarter(tstn, 1 - par, b // QB)
                        else:
                            # pooling interleaved into layer 2's edge pass
                            h3t = psA.tile([128, HID], F32, tag="psA")
                            nc.tensor.matmul(
                                h3t[:], hT[:, b * 128:(b + 1) * 128],
                                ident64_sb[:], start=True, stop=True)
                            h3s = epsp.tile([128, HID], F16, tag="h3s")
                            nc.vector.tensor_copy(out=h3s[:], in_=h3t[:])
                            nc.tensor.matmul(
                                pool_ps[:], seg_sb[:, b, :], h3s[:],
                                start=(b == 0), stop=(b == NBLK - 1))

            pool_sb = epsp.tile([B, HID], F32, tag="poolsb")
            nc.vector.tensor_copy(out=pool_sb[:], in_=pool_ps[:])
            nc.sync.dma_start(out=pool_in[:], in_=pool_sb[:])
            nc.gpsimd.collective_compute(
                "AllReduce", mybir.AluOpType.add, replica_groups=groups,
                ins=[pool_in[:]], outs=[pool_out[:]])
            pooled = epsp.tile([B, HID], F32, tag="pooled")
            nc.sync.dma_start(out=pooled[:], in_=pool_out[:])
            pm = epsp.tile([B, HID], F32, tag="pm")
            nc.vector.tensor_tensor(
                out=pm[:], in0=pooled[:],
                in1=invcnt_sb[:].to_broadcast([B, HID]),
                op=mybir.AluOpType.mult)
            ppT = psA.tile([HID, B], F32, tag="psA")
            nc.tensor.matmul(ppT[:], pm[:], identB_sb[:], start=True, stop=True)
            pT = epsp.tile([HID, B], F32, tag="pT")
            nc.vector.tensor_copy(out=pT[:], in_=ppT[:])
            z1p = psB.tile([HID // 2, B], F32, tag="psB")
            nc.tensor.matmul(z1p[:], mlpw_sb[:, :HID // 2], pT[:], start=True, stop=True)
            z1 = epsp.tile([HID // 2, B], F32, tag="z1")
            nc.scalar.activation(
                out=z1[:], in_=z1p[:],
                func=mybir.ActivationFunctionType.Relu, bias=bm_sb[:, 0:1])
            z2p = psB.tile([1, B], F32, tag="psB")
            nc.tensor.matmul(
                z2p[:], mlpw_sb[:HID // 2, HID // 2:HID // 2 + 1], z1[:],
                start=True, stop=True)
            zf = epsp.tile([1, B], F32, tag="zf")
            nc.vector.tensor_tensor(
                out=zf[:], in0=z2p[:],
                in1=bm_sb[0:1, 1:2].to_broadcast([1, B]),
                op=mybir.AluOpType.add)
            nc.sync.dma_start(out=z[:], in_=zf[:])

    nc.finalize()
    return nc


_CACHE = {}


def kernel(x, edge_index, batch, W0, b0, W1, b1, W2, b2, Wm1, bm1, Wm2, bm2,
           trace=False):
    x = np.asarray(x, np.float32)
    dis, Cm, idx_stream, dl_stream, perm = _preprocess(
        np.asarray(edge_index), np.asarray(batch))
    C = Cm[0]
    Ctot = int(C.sum())
    S = Ctot * 128

    gid = np.asarray(batch, np.int64)
    cnts = np.bincount(gid, minlength=B).astype(np.float32)
    packed = perm

    xp = np.zeros((NCORES, IN_C, NLOC), np.float16)
    xp[packed // NLOC, :, packed % NLOC] = x.astype(np.float16)
    disp = np.zeros((NCORES, NLOC), np.float32)
    disp[packed // NLOC, packed % NLOC] = dis
    seg = np.zeros((NCORES, NLOC, B), np.float16)
    seg[packed // NLOC, packed % NLOC, gid] = 1.0

    iota = np.tile(np.arange(256, dtype=np.float16)[None, :], (128, 1))
    in_maps = []
    for k in range(NCORES):
        in_maps.append(dict(
            xT=xp[k],
            disN=disp[k][:, None].copy(),
            disT=disp[k][None, :].copy(),
            W0=np.asarray(W0, np.float16),
            W1=np.asarray(W1, np.float16),
            W2=np.asarray(W2, np.float16),
            b0=np.asarray(b0, np.float32)[:, None],
            b1=np.asarray(b1, np.float32)[:, None],
            b2=np.asarray(b2, np.float32)[:, None],
            Wm1=np.asarray(Wm1, np.float32),
            bm1=np.asarray(bm1, np.float32)[:, None],
            Wm2=np.asarray(Wm2, np.float32),
            bm2=np.asarray(bm2, np.float32).reshape(1, 1),
            idx16=np.tile(idx_stream[k].reshape(S // 16, 16).T, (8, 1)).copy(),
            dl16=dl_stream[k].reshape(Ctot, 128).T.copy(),
            segsel=seg[k],
            invcnt=(1.0 / np.maximum(cnts, 1.0)).astype(np.float32)[:, None],
            iota128=iota,
            ident64=np.eye(HID, dtype=np.float16),
            ident128=np.eye(128, dtype=np.float16),
            identB=np.eye(B, dtype=np.float32),
        ))

    ckey = C.tobytes() + Cm[1].tobytes() + Cm[2].tobytes() + Cm[3].tobytes()
    if ckey not in _CACHE:
        _CACHE[ckey] = _build_nc(Cm)
    nc = _CACHE[ckey]

    res = run_bass_kernel_spmd(nc, in_maps, list(range(NCORES)), trace=trace)
    out = res.results[0]["z"].reshape(B, 1).astype(np.float32)
    if trace:
        return out, res
    return out


# revision 24
# speedup vs baseline: 1.2287x; 1.0313x over previous
"""FootballGCN (3x GCNConv + mean-pool + MLP) on 8 TRN2 NeuronCores.

Self-contained: takes full inputs, shards internally, runs a Bass/Tile SPMD
kernel via run_bass_kernel_spmd, returns the full (B, 1) output.

Strategy (dst-sharded message passing, feature-major on-chip layout):
  - nodes packed into 8 cores x 12800 local slots (12500 real + pad)
  - table t = dis * (h @ W) built per 128-node block; layer l+1's table is
    computed incrementally DURING layer l's edge pass (hT blocks land
    progressively), and each finished quarter is immediately staged +
    AllGathered into the other t_full parity, hiding table+collective
    behind the previous layer's edge pass
  - edge pass: per superblock (SB dst blocks) x 4 source slabs (int16 index
    range; slab g is Q7-pair-g's SWDGE queue so desc-gen overlaps 4-way),
    dma_gather 256B rows; selection matrix built on DVE via
    is_equal(dst_local, iota); PE matmul (lhsT=msg, rhs=Sel) accumulates
    per-block agg[64, 128] in PSUM, seeded with the self-loop term t^T @ I;
    epilogue relu(dis*agg + b) on DVE+ACT
  - pooling interleaved into layer 2's edge pass; AllReduce; tiny MLP
"""
import numpy as np

import concourse.bass as bass
import concourse.mybir as mybir
import concourse.tile as tile
from concourse import bacc as bacc_mod
from concourse.bass_utils import run_bass_kernel_spmd

F16 = mybir.dt.float16
F32 = mybir.dt.float32
I16 = mybir.dt.int16

# ---- problem dims (hardcoded per spec) ----
N = 100000
E = 3200000
B = 128
IN_C, HID = 128, 64
NCORES = 8
NREAL = 12500
NBLK = 100                   # blocks per core (multiple of 4 for slab split)
NLOC = NBLK * 128            # 12800
NTOT = NCORES * NLOC         # 102400
NGRP = 4
QB = NBLK // NGRP            # 25 blocks per quarter
QLOC = QB * 128              # 3200 rows per quarter
GRP = NCORES * QLOC          # 25600 rows per slab (int16-safe)
SB = 2                       # blocks per superblock (must divide NBLK)
NSUP = NBLK // SB
NSWQ = 4                     # SWDGE queues (desc-gen Q7 core pairs)


def _preprocess(edge_index, batch):
    # self-loops are NOT streamed as edges; they are applied on-chip via an
    # identity-seeded PSUM matmul. deg still counts them (reference adds
    # loops before computing symmetric normalization).
    src_g = np.asarray(edge_index[0], np.int64)
    dst_g = np.asarray(edge_index[1], np.int64)

    deg = (np.bincount(dst_g, minlength=N) + 1).astype(np.float64)
    dis = (1.0 / np.sqrt(np.maximum(deg, 1.0))).astype(np.float32)

    # snake-balanced node -> packed-slot assignment: deal nodes (sorted by
    # in-degree desc) across all NCORES*NBLK blocks so per-(core,blk,grp)
    # edge counts equalize -> smaller chunk-count C (less Q7 desc-gen).
    nblk_all = NCORES * NBLK
    order = np.argsort(-deg, kind="stable")
    pos = np.arange(N)
    cyc, r = pos // nblk_all, pos % nblk_all
    blk_of = np.where(cyc % 2 == 0, r, nblk_all - 1 - r)
    rank_of = cyc
    perm = np.empty(N, dtype=np.int64)
    gblk = blk_of
    perm[order] = (gblk // NBLK) * NLOC + (gblk % NBLK) * 128 + rank_of
    assert rank_of.max() < 128

    src_p = perm[src_g]
    dst_p = perm[dst_g]

    core = dst_p // NLOC
    blk = (dst_p % NLOC) // 128
    dcol = dst_p % 128
    # source slab: quarter q of the owning core; slab-local index
    src_core = src_p // NLOC
    src_loc = src_p % NLOC
    grp = src_loc // QLOC
    lidx = src_core * QLOC + src_loc % QLOC

    order = np.lexsort((dcol, blk, grp, blk // SB, core))
    core_s, blk_s, grp_s, dcol_s, lidx_s = (
        core[order], blk[order], grp[order], dcol[order], lidx[order])

    sup_s = blk_s // SB
    # per (core, sup, g) counts; A = first block of sup
    key_sg = (core_s * NSUP + sup_s) * NGRP + grp_s
    cnt_sg = np.bincount(key_sg, minlength=NCORES * NSUP * NGRP)
    cnt_sg = cnt_sg.reshape(NCORES, NSUP, NGRP)
    isA = (blk_s % SB) == 0
    cnt_a = np.bincount(key_sg[isA], minlength=NCORES * NSUP * NGRP)
    cnt_a = cnt_a.reshape(NCORES, NSUP, NGRP)

    n_sg = np.maximum(cnt_sg.max(axis=0), 1)       # (NSUP, NGRP)
    C = np.ceil(n_sg / 128).astype(np.int64)
    cA = np.maximum(np.ceil(cnt_a / 128).astype(np.int64).max(axis=0), 1)
    cA = np.minimum(cA, C)
    cB0 = np.minimum((cnt_a // 128).min(axis=0), C - 1)
    Ctot = int(C.sum())
    S = Ctot * 128

    # slot offsets in stream order (sup, g, chunks)
    off_sg = np.zeros((NSUP, NGRP), dtype=np.int64)
    acc = 0
    for s in range(NSUP):
        for g in range(NGRP):
            off_sg[s, g] = acc
            acc += C[s, g] * 128
    assert acc == S

    idx_stream = np.full((NCORES, S), -1, dtype=np.int16)
    dl_stream = np.full((NCORES, S), -1.0, dtype=np.float16)

    EA = len(key_sg)
    is_start = np.ones(EA, dtype=bool)
    is_start[1:] = key_sg[1:] != key_sg[:-1]
    run_start_idx = np.flatnonzero(is_start)
    run_id = np.cumsum(is_start) - 1
    run_pos = np.arange(EA) - run_start_idx[run_id]
    slot = off_sg[sup_s, grp_s] + run_pos
    idx_stream[core_s, slot] = lidx_s.astype(np.int16)
    dl_stream[core_s, slot] = ((blk_s % SB) * 128 + dcol_s).astype(np.float16)

    # all pad slots gather row 0 (valid, dl=-1 keeps Sel at 0) so every mt
    # lane is written — a skipped lane would leave stale SBUF that can be
    # Inf/NaN, and 0 * Inf = NaN through the Sel matmul
    idx_stream[idx_stream < 0] = 0

    return dis, (C, cA, cB0, n_sg), idx_stream, dl_stream, perm


def _build_nc(Cm):
    C, cA, cB0, n_sg = Cm
    NGRPL = NGRP
    Ctot = int(C.sum())
    S = Ctot * 128
    C_sup_g = C
    off_sup = np.zeros(NSUP + 1, dtype=np.int64)
    for s in range(NSUP):
        off_sup[s + 1] = off_sup[s] + C[s].sum()

    # multiple SWDGE queues: dma_gather desc-gen for queue q runs on Q7 core
    # pair (2q, 2q+1), so different queues' descriptor generation overlaps
    nc = bacc_mod.Bacc(num_swdge_queues=NSWQ)

    xT = nc.declare_dram_parameter("xT", [IN_C, NLOC], F16, isOutput=False)
    disN = nc.declare_dram_parameter("disN", [NLOC, 1], F32, isOutput=False)
    disT = nc.declare_dram_parameter("disT", [1, NLOC], F32, isOutput=False)
    W0 = nc.declare_dram_parameter("W0", [IN_C, HID], F16, isOutput=False)
    W1 = nc.declare_dram_parameter("W1", [HID, HID], F16, isOutput=False)
    W2 = nc.declare_dram_parameter("W2", [HID, HID], F16, isOutput=False)
    b0 = nc.declare_dram_parameter("b0", [HID, 1], F32, isOutput=False)
    b1 = nc.declare_dram_parameter("b1", [HID, 1], F32, isOutput=False)
    b2 = nc.declare_dram_parameter("b2", [HID, 1], F32, isOutput=False)
    Wm1 = nc.declare_dram_parameter("Wm1", [HID, HID // 2], F32, isOutput=False)
    bm1 = nc.declare_dram_parameter("bm1", [HID // 2, 1], F32, isOutput=False)
    Wm2 = nc.declare_dram_parameter("Wm2", [HID // 2, 1], F32, isOutput=False)
    bm2 = nc.declare_dram_parameter("bm2", [1, 1], F32, isOutput=False)
    idx16 = nc.declare_dram_parameter("idx16", [128, S // 16], I16, isOutput=False)
    dl16 = nc.declare_dram_parameter("dl16", [128, Ctot], F16, isOutput=False)
    segsel = nc.declare_dram_parameter("segsel", [NLOC, B], F16, isOutput=False)
    invcnt = nc.declare_dram_parameter("invcnt", [B, 1], F32, isOutput=False)
    iota128 = nc.declare_dram_parameter("iota128", [128, 256], F16, isOutput=False)
    ident64 = nc.declare_dram_parameter("ident64", [HID, HID], F16, isOutput=False)
    ident128 = nc.declare_dram_parameter("ident128", [128, 128], F16, isOutput=False)
    identB = nc.declare_dram_parameter("identB", [B, B], F32, isOutput=False)
    z = nc.declare_dram_parameter("z", [1, B], F32, isOutput=True)

    t_loc_q = [nc.dram_tensor(f"t_loc{q}", [QLOC, 128], F16)
               for q in range(NGRP)]
    # double-buffered by layer parity: layer l reads parity l%2 while layer
    # l+1's AllGathers (issued mid-edge-pass) write parity (l+1)%2
    t_full_q = [[nc.dram_tensor(f"t_full{p}_{q}", [GRP, 128], F16,
                                addr_space="Shared")
                 for q in range(NGRP)] for p in range(2)]
    pool_in = nc.dram_tensor("pool_in", [B, HID], F32)
    pool_out = nc.dram_tensor("pool_out", [B, HID], F32, addr_space="Shared")

    groups = [list(range(NCORES))]

    with tile.TileContext(nc) as tc:
        with (
            tc.tile_pool(name="const", bufs=1) as constp,
            tc.tile_pool(name="hT", bufs=1) as hTp,
            tc.tile_pool(name="tstage", bufs=2) as tstp,
            tc.tile_pool(name="xblk", bufs=2) as xblkp,
            tc.tile_pool(name="idx", bufs=3) as idxp,
            tc.tile_pool(name="msg", bufs=3) as msgp,
            tc.tile_pool(name="sel", bufs=2) as selp,
            tc.tile_pool(name="eps", bufs=3) as epsp,
            tc.tile_pool(name="psA", bufs=2, space="PSUM") as psA,
            tc.tile_pool(name="psB", bufs=2, space="PSUM") as psB,
            tc.tile_pool(name="psPool", bufs=1, space="PSUM") as psPoolp,
        ):
            disN_sb = constp.tile([128, NBLK], F32)
            nc.sync.dma_start(
                out=disN_sb[:], in_=disN.rearrange("(b p) o -> p (b o)", p=128))
            W0_sb = constp.tile([IN_C, HID], F16)
            nc.sync.dma_start(out=W0_sb[:], in_=W0[:])
            W1_sb = constp.tile([HID, HID], F16)
            nc.sync.dma_start(out=W1_sb[:], in_=W1[:])
            W2_sb = constp.tile([HID, HID], F16)
            nc.sync.dma_start(out=W2_sb[:], in_=W2[:])
            bias_sb = constp.tile([HID, 3], F32)
            for i, bb in enumerate([b0, b1, b2]):
                nc.sync.dma_start(out=bias_sb[:, i:i + 1], in_=bb[:])
            iota_sb = constp.tile([128, 256], F16)
            nc.sync.dma_start(out=iota_sb[:], in_=iota128[:])
            dl_sb = constp.tile([128, Ctot], F16)
            nc.sync.dma_start(out=dl_sb[:], in_=dl16[:])
            ident128_sb = constp.tile([128, 128], F16)
            nc.sync.dma_start(out=ident128_sb[:], in_=ident128[:])

            hT = hTp.tile([HID, NLOC], F16, tag="hT")
            pool_ps = psPoolp.tile([B, HID], F32)

            def table_mult(tstc, b, pt):
                nc.vector.tensor_tensor(
                    out=tstc[:, b, :], in0=pt[:],
                    in1=disN_sb[:, b:b + 1].to_broadcast([128, HID]),
                    op=mybir.AluOpType.mult)

            def stage_quarter(tstc, par, q):
                # duplicate the 64-wide table rows into 256B [t|t] rows via
                # two DMAs, then AllGather this slab into parity `par`
                tl = t_loc_q[q].rearrange("(b p) (h e) -> p b h e",
                                          p=128, h=2)
                src = tstc[:, q * QB:(q + 1) * QB, :].unsqueeze(2)
                for half in range(2):
                    nc.sync.dma_start(out=tl[:, :, half:half + 1, :], in_=src)
                nc.gpsimd.collective_compute(
                    "AllGather", mybir.AluOpType.bypass,
                    replica_groups=groups,
                    ins=[t_loc_q[q][:]], outs=[t_full_q[par][q][:]])

            # layer-0 table prologue (parity 0)
            tst0 = tstp.tile([128, NBLK, HID], F16, tag="tstage")
            tsts = [tst0]
            xq = None
            for b in range(NBLK):
                if b % QB == 0:
                    xq = xblkp.tile([IN_C, QLOC], F16, tag="xq")
                    nc.sync.dma_start(
                        out=xq[:],
                        in_=xT[:, (b // QB) * QLOC:(b // QB + 1) * QLOC])
                pt = psA.tile([128, HID], F32, tag="psA")
                nc.tensor.matmul(
                    pt[:], xq[:, (b % QB) * 128:(b % QB + 1) * 128],
                    W0_sb[:], start=True, stop=True)
                table_mult(tsts[0], b, pt)
                if (b + 1) % QB == 0:
                    stage_quarter(tsts[0], 0, b // QB)

            # late constants: not needed until layer 2 / the tail, so load
            # them behind the layer-0 edge pass instead of ahead of it
            ident64_sb = constp.tile([HID, HID], F16)
            nc.sync.dma_start(out=ident64_sb[:], in_=ident64[:])
            identB_sb = constp.tile([B, B], F32)
            nc.sync.dma_start(out=identB_sb[:], in_=identB[:])
            invcnt_sb = constp.tile([B, 1], F32)
            nc.sync.dma_start(out=invcnt_sb[:], in_=invcnt[:])
            mlpw_sb = constp.tile([HID, HID // 2 + 1], F32)
            nc.sync.dma_start(out=mlpw_sb[:, :HID // 2], in_=Wm1[:])
            nc.sync.dma_start(out=mlpw_sb[:HID // 2, HID // 2:], in_=Wm2[:])
            bm_sb = constp.tile([HID // 2, 2], F32)
            nc.sync.dma_start(out=bm_sb[:, 0:1], in_=bm1[:])
            nc.sync.dma_start(out=bm_sb[0:1, 1:2], in_=bm2[:])

            for l in range(3):
                par = l % 2
                tstc = tsts[l]
                if l < 2:
                    tstn = tstp.tile([128, NBLK, HID], F16, tag="tstage")
                    tsts.append(tstn)
                    Wn_sb = W1_sb if l == 0 else W2_sb

                for s in range(NSUP):
                    sup_slot0 = int(off_sup[s]) * 128
                    sup_slots = int(C_sup_g[s].sum()) * 128
                    disTB = idxp.tile([HID, SB * 128], F32, tag="disTB")
                    nc.sync.dma_start(
                        out=disTB[:].unsqueeze(1),
                        in_=disT[:, s * SB * 128:(s + 1) * SB * 128]
                        .partition_broadcast(HID))
                    idxb = idxp.tile([128, sup_slots // 16], I16, tag="idx")
                    nc.sync.dma_start(
                        out=idxb[:],
                        in_=idx16[:, sup_slot0 // 16:(sup_slot0 + sup_slots) // 16])
                    mts, selAs, selBs = [], [], []
                    goff = 0
                    for g in range(NGRPL):
                        cg = int(C[s, g])
                        ca, cb0 = int(cA[s, g]), int(cB0[s, g])
                        dl0 = int(off_sup[s]) + goff
                        selA = selp.tile([128, ca, 128], F16, tag=f"selA{g}")
                        nc.vector.tensor_tensor(
                            out=selA[:],
                            in0=dl_sb[:, dl0:dl0 + ca]
                            .unsqueeze(2).to_broadcast([128, ca, 128]),
                            in1=iota_sb[:, 0:128].unsqueeze(1)
                            .to_broadcast([128, ca, 128]),
                            op=mybir.AluOpType.is_equal)
                        nb = cg - cb0
                        selB = selp.tile([128, nb, 128], F16, tag=f"selB{g}")
                        nc.vector.tensor_tensor(
                            out=selB[:],
                            in0=dl_sb[:, dl0 + cb0:dl0 + cg]
                            .unsqueeze(2).to_broadcast([128, nb, 128]),
                            in1=iota_sb[:, 128:256].unsqueeze(1)
                            .to_broadcast([128, nb, 128]),
                            op=mybir.AluOpType.is_equal)
                        selAs.append(selA)
                        selBs.append(selB)
                        mt = msgp.tile([128, cg, 128], F16, tag=f"msg{g}")
                        nidx = cg * 128
                        nc.gpsimd.dma_gather(
                            out_ap=mt[:],
                            in_ap=t_full_q[par][g][:, :],
                            idxs_ap=idxb[:, (goff * 128) // 16:
                                         (goff * 128 + nidx) // 16],
                            num_idxs=nidx, num_idxs_reg=nidx,
                            elem_size=128,
                            single_packet=False, queue_num=g % NSWQ)
                        mts.append(mt)
                        goff += cg
                    for bi in range(SB):
                        b = s * SB + bi
                        agg = psB.tile([HID, 128], F32, tag="psB")
                        if bi == 0:
                            njob = int(cA[s].sum())
                        else:
                            njob = int((C[s] - cB0[s]).sum())
                        # seed with the self-loop term: agg = t_blk^T
                        # (epilogue's *dis makes it dis^2 * (hW), the
                        # reference's self-loop contribution)
                        nc.tensor.matmul(
                            agg[:], tstc[:, b, :], ident128_sb[:],
                            start=True, stop=False)
                        j = 0
                        for g in range(NGRPL):
                            if bi == 0:
                                rng_c = range(int(cA[s, g]))
                                stile, soff = selAs[g], 0
                            else:
                                rng_c = range(int(cB0[s, g]), int(C[s, g]))
                                stile, soff = selBs[g], int(cB0[s, g])
                            for c in rng_c:
                                nc.tensor.matmul(
                                    agg[:], mts[g][:, c, 0:HID],
                                    stile[:, c - soff, :],
                                    start=False, stop=(j == njob - 1))
                                j += 1
                        ep = epsp.tile([HID, 128], F32, tag="eps")
                        nc.vector.tensor_tensor(
                            out=ep[:], in0=agg[:],
                            in1=disTB[:, bi * 128:(bi + 1) * 128],
                            op=mybir.AluOpType.mult)
                        nc.scalar.activation(
                            out=hT[:, b * 128:(b + 1) * 128], in_=ep[:],
                            func=mybir.ActivationFunctionType.Relu,
                            bias=bias_sb[:, l:l + 1])
                        if l < 2:
                            # next layer's table block b, immediately after
                            # its hT lands; each finished quarter is staged
                            # and AllGathered into the other parity while
                            # this layer's edge pass continues
                            pt = psA.tile([128, HID], F32, tag="psA")
                            nc.tensor.matmul(
                                pt[:], hT[:, b * 128:(b + 1) * 128],
                                Wn_sb[:], start=True, stop=True)
                            table_mult(tstn, b, pt)
                            if (b + 1) % QB == 0:
                                stage_quarter(tstn, 1 - par, b // QB)
                        else:
                            # pooling interleaved into layer 2's edge pass;
                            # segsel streamed per quarter to save SBUF
                            if b % QB == 0:
                                segq = xblkp.tile([128, QB, B], F16,
                                                  tag="segq")
                                nc.sync.dma_start(
                                    out=segq[:],
                                    in_=segsel[(b // QB) * QLOC:
                                               (b // QB + 1) * QLOC, :]
                                    .rearrange("(b p) g -> p b g", p=128))
                            h3t = psA.tile([128, HID], F32, tag="psA")
                            nc.tensor.matmul(
                                h3t[:], hT[:, b * 128:(b + 1) * 128],
                                ident64_sb[:], start=True, stop=True)
                            h3s = epsp.tile([128, HID], F16, tag="h3s")
                            nc.vector.tensor_copy(out=h3s[:], in_=h3t[:])
                            nc.tensor.matmul(
                                pool_ps[:], segq[:, b % QB, :], h3s[:],
                                start=(b == 0), stop=(b == NBLK - 1))

            pool_sb = epsp.tile([B, HID], F32, tag="poolsb")
            nc.vector.tensor_copy(out=pool_sb[:], in_=pool_ps[:])
            nc.sync.dma_start(out=pool_in[:], in_=pool_sb[:])
            nc.gpsimd.collective_compute(
                "AllReduce", mybir.AluOpType.add, replica_groups=groups,
                ins=[pool_in[:]], outs=[pool_out[:]])
            pooled = epsp.tile([B, HID], F32, tag="pooled")
            nc.sync.dma_start(out=pooled[:], in_=pool_out[:])
            pm = epsp.tile([B, HID], F32, tag="pm")
            nc.vector.tensor_tensor(
                out=pm[:], in0=pooled[:],
                in1=invcnt_sb[:].to_broadcast([B, HID]),
                op=mybir.AluOpType.mult)
            ppT = psA.tile([HID, B], F32, tag="psA")
            nc.tensor.matmul(ppT[:], pm[:], identB_sb[:], start=True, stop=True)
            pT = epsp.tile([HID, B], F32, tag="pT")
            nc.vector.tensor_copy(out=pT[:], in_=ppT[:])
            z1p = psB.tile([HID // 2, B], F32, tag="psB")
            nc.tensor.matmul(z1p[:], mlpw_sb[:, :HID // 2], pT[:], start=True, stop=True)
            z1 = epsp.tile([HID // 2, B], F32, tag="z1")
            nc.scalar.activation(
                out=z1[:], in_=z1p[:],
                func=mybir.ActivationFunctionType.Relu, bias=bm_sb[:, 0:1])
            z2p = psB.tile([1, B], F32, tag="psB")
            nc.tensor.matmul(
                z2p[:], mlpw_sb[:HID // 2, HID // 2:HID // 2 + 1], z1[:],
                start=True, stop=True)
            zf = epsp.tile([1, B], F32, tag="zf")
            nc.vector.tensor_tensor(
                out=zf[:], in0=z2p[:],
                in1=bm_sb[0:1, 1:2].to_broadcast([1, B]),
                op=mybir.AluOpType.add)
            nc.sync.dma_start(out=z[:], in_=zf[:])

    nc.finalize()
    return nc


_CACHE = {}


def kernel(x, edge_index, batch, W0, b0, W1, b1, W2, b2, Wm1, bm1, Wm2, bm2,
           trace=False):
    x = np.asarray(x, np.float32)
    dis, Cm, idx_stream, dl_stream, perm = _preprocess(
        np.asarray(edge_index), np.asarray(batch))
    C = Cm[0]
    Ctot = int(C.sum())
    S = Ctot * 128

    gid = np.asarray(batch, np.int64)
    cnts = np.bincount(gid, minlength=B).astype(np.float32)
    packed = perm

    xp = np.zeros((NCORES, IN_C, NLOC), np.float16)
    xp[packed // NLOC, :, packed % NLOC] = x.astype(np.float16)
    disp = np.zeros((NCORES, NLOC), np.float32)
    disp[packed // NLOC, packed % NLOC] = dis
    seg = np.zeros((NCORES, NLOC, B), np.float16)
    seg[packed // NLOC, packed % NLOC, gid] = 1.0

    iota = np.tile(np.arange(256, dtype=np.float16)[None, :], (128, 1))
    in_maps = []
    for k in range(NCORES):
        in_maps.append(dict(
            xT=xp[k],
            disN=disp[k][:, None].copy(),
            disT=disp[k][None, :].copy(),
            W0=np.asarray(W0, np.float16),
            W1=np.asarray(W1, np.float16),
            W2=np.asarray(W2, np.float16),
            b0=np.asarray(b0, np.float32)[:, None],
            b1=np.asarray(b1, np.float32)[:, None],
            b2=np.asarray(b2, np.float32)[:, None],
            Wm1=np.asarray(Wm1, np.float32),
            bm1=np.asarray(bm1, np.float32)[:, None],
            Wm2=np.asarray(Wm2, np.float32),
            bm2=np.asarray(bm2, np.float32).reshape(1, 1),
            idx16=np.tile(idx_stream[k].reshape(S // 16, 16).T, (8, 1)).copy(),
            dl16=dl_stream[k].reshape(Ctot, 128).T.copy(),
            segsel=seg[k],
            invcnt=(1.0 / np.maximum(cnts, 1.0)).astype(np.float32)[:, None],
            iota128=iota,
            ident64=np.eye(HID, dtype=np.float16),
            ident128=np.eye(128, dtype=np.float16),
            identB=np.eye(B, dtype=np.float32),
        ))

    ckey = C.tobytes() + Cm[1].tobytes() + Cm[2].tobytes() + Cm[3].tobytes()
    if ckey not in _CACHE:
        _CACHE[ckey] = _build_nc(Cm)
    nc = _CACHE[ckey]

    res = run_bass_kernel_spmd(nc, in_maps, list(range(NCORES)), trace=trace)
    out = res.results[0]["z"].reshape(B, 1).astype(np.float32)
    if trace:
        return out, res
    return out


# revision 26
# speedup vs baseline: 1.2423x; 1.0111x over previous
"""FootballGCN (3x GCNConv + mean-pool + MLP) on 8 TRN2 NeuronCores.

Self-contained: takes full inputs, shards internally, runs a Bass/Tile SPMD
kernel via run_bass_kernel_spmd, returns the full (B, 1) output.

Strategy (dst-sharded message passing, feature-major on-chip layout):
  - nodes packed into 8 cores x 12800 local slots (12500 real + pad)
  - table t = dis * (h @ W) built per 128-node block; layer l+1's table is
    computed incrementally DURING layer l's edge pass (hT blocks land
    progressively), and each finished quarter is immediately staged +
    AllGathered into the other t_full parity, hiding table+collective
    behind the previous layer's edge pass
  - edge pass: per superblock (SB dst blocks) x 4 source slabs (int16 index
    range; slab g is Q7-pair-g's SWDGE queue so desc-gen overlaps 4-way),
    dma_gather 256B rows; selection matrix built on DVE via
    is_equal(dst_local, iota); PE matmul (lhsT=msg, rhs=Sel) accumulates
    per-block agg[64, 128] in PSUM, seeded with the self-loop term t^T @ I;
    epilogue relu(dis*agg + b) on DVE+ACT
  - pooling interleaved into layer 2's edge pass; AllReduce; tiny MLP
"""
import numpy as np

import concourse.bass as bass
import concourse.mybir as mybir
import concourse.tile as tile
from concourse import bacc as bacc_mod
from concourse.bass_utils import run_bass_kernel_spmd

F16 = mybir.dt.float16
F32 = mybir.dt.float32
I16 = mybir.dt.int16

# ---- problem dims (hardcoded per spec) ----
N = 100000
E = 3200000
B = 128
IN_C, HID = 128, 64
NCORES = 8
NREAL = 12500
NBLK = 100                   # blocks per core (multiple of 4 for slab split)
NLOC = NBLK * 128            # 12800
NTOT = NCORES * NLOC         # 102400
NGRP = 4
QB = NBLK // NGRP            # 25 blocks per quarter
QLOC = QB * 128              # 3200 rows per quarter
GRP = NCORES * QLOC          # 25600 rows per slab (int16-safe)
SB = 2                       # blocks per superblock (must divide NBLK)
NSUP = NBLK // SB
NSWQ = 4                     # SWDGE queues (desc-gen Q7 core pairs)


def _preprocess(edge_index, batch):
    # self-loops are NOT streamed as edges; they are applied on-chip via an
    # identity-seeded PSUM matmul. deg still counts them (reference adds
    # loops before computing symmetric normalization).
    src_g = np.asarray(edge_index[0], np.int64)
    dst_g = np.asarray(edge_index[1], np.int64)

    deg = (np.bincount(dst_g, minlength=N) + 1).astype(np.float64)
    dis = (1.0 / np.sqrt(np.maximum(deg, 1.0))).astype(np.float32)

    # snake-balanced node -> packed-slot assignment: deal nodes (sorted by
    # in-degree desc) across all NCORES*NBLK blocks so per-(core,blk,grp)
    # edge counts equalize -> smaller chunk-count C (less Q7 desc-gen).
    nblk_all = NCORES * NBLK
    order = np.argsort(-deg, kind="stable")
    pos = np.arange(N)
    cyc, r = pos // nblk_all, pos % nblk_all
    blk_of = np.where(cyc % 2 == 0, r, nblk_all - 1 - r)
    rank_of = cyc
    perm = np.empty(N, dtype=np.int64)
    gblk = blk_of
    perm[order] = (gblk // NBLK) * NLOC + (gblk % NBLK) * 128 + rank_of
    assert rank_of.max() < 128

    src_p = perm[src_g]
    dst_p = perm[dst_g]

    core = dst_p // NLOC
    blk = (dst_p % NLOC) // 128
    dcol = dst_p % 128
    # source slab: quarter q of the owning core; slab-local index
    src_core = src_p // NLOC
    src_loc = src_p % NLOC
    grp = src_loc // QLOC
    lidx = src_core * QLOC + src_loc % QLOC

    order = np.lexsort((dcol, blk, grp, blk // SB, core))
    core_s, blk_s, grp_s, dcol_s, lidx_s = (
        core[order], blk[order], grp[order], dcol[order], lidx[order])

    sup_s = blk_s // SB
    # per (core, sup, g) counts; A = first block of sup
    key_sg = (core_s * NSUP + sup_s) * NGRP + grp_s
    cnt_sg = np.bincount(key_sg, minlength=NCORES * NSUP * NGRP)
    cnt_sg = cnt_sg.reshape(NCORES, NSUP, NGRP)
    isA = (blk_s % SB) == 0
    cnt_a = np.bincount(key_sg[isA], minlength=NCORES * NSUP * NGRP)
    cnt_a = cnt_a.reshape(NCORES, NSUP, NGRP)

    n_sg = np.maximum(cnt_sg.max(axis=0), 1)       # (NSUP, NGRP)
    C = np.ceil(n_sg / 128).astype(np.int64)
    cA = np.maximum(np.ceil(cnt_a / 128).astype(np.int64).max(axis=0), 1)
    cA = np.minimum(cA, C)
    cB0 = np.minimum((cnt_a // 128).min(axis=0), C - 1)
    Ctot = int(C.sum())
    S = Ctot * 128

    # slot offsets in stream order (sup, g, chunks)
    off_sg = np.zeros((NSUP, NGRP), dtype=np.int64)
    acc = 0
    for s in range(NSUP):
        for g in range(NGRP):
            off_sg[s, g] = acc
            acc += C[s, g] * 128
    assert acc == S

    idx_stream = np.full((NCORES, S), -1, dtype=np.int16)
    dl_stream = np.full((NCORES, S), -1.0, dtype=np.float16)

    EA = len(key_sg)
    is_start = np.ones(EA, dtype=bool)
    is_start[1:] = key_sg[1:] != key_sg[:-1]
    run_start_idx = np.flatnonzero(is_start)
    run_id = np.cumsum(is_start) - 1
    run_pos = np.arange(EA) - run_start_idx[run_id]
    slot = off_sg[sup_s, grp_s] + run_pos
    idx_stream[core_s, slot] = lidx_s.astype(np.int16)
    dl_stream[core_s, slot] = ((blk_s % SB) * 128 + dcol_s).astype(np.float16)

    # all pad slots gather row 0 (valid, dl=-1 keeps Sel at 0) so every mt
    # lane is written — a skipped lane would leave stale SBUF that can be
    # Inf/NaN, and 0 * Inf = NaN through the Sel matmul
    idx_stream[idx_stream < 0] = 0

    return dis, (C, cA, cB0, n_sg), idx_stream, dl_stream, perm


def _build_nc(Cm):
    C, cA, cB0, n_sg = Cm
    NGRPL = NGRP
    Ctot = int(C.sum())
    S = Ctot * 128
    C_sup_g = C
    off_sup = np.zeros(NSUP + 1, dtype=np.int64)
    for s in range(NSUP):
        off_sup[s + 1] = off_sup[s] + C[s].sum()

    # multiple SWDGE queues: dma_gather desc-gen for queue q runs on Q7 core
    # pair (2q, 2q+1), so different queues' descriptor generation overlaps
    nc = bacc_mod.Bacc(num_swdge_queues=NSWQ)

    xT = nc.declare_dram_parameter("xT", [IN_C, NLOC], F16, isOutput=False)
    disN = nc.declare_dram_parameter("disN", [NLOC, 1], F32, isOutput=False)
    disT = nc.declare_dram_parameter("disT", [1, NLOC], F32, isOutput=False)
    W0 = nc.declare_dram_parameter("W0", [IN_C, HID], F16, isOutput=False)
    W1 = nc.declare_dram_parameter("W1", [HID, HID], F16, isOutput=False)
    W2 = nc.declare_dram_parameter("W2", [HID, HID], F16, isOutput=False)
    b0 = nc.declare_dram_parameter("b0", [HID, 1], F32, isOutput=False)
    b1 = nc.declare_dram_parameter("b1", [HID, 1], F32, isOutput=False)
    b2 = nc.declare_dram_parameter("b2", [HID, 1], F32, isOutput=False)
    Wm1 = nc.declare_dram_parameter("Wm1", [HID, HID // 2], F32, isOutput=False)
    bm1 = nc.declare_dram_parameter("bm1", [HID // 2, 1], F32, isOutput=False)
    Wm2 = nc.declare_dram_parameter("Wm2", [HID // 2, 1], F32, isOutput=False)
    bm2 = nc.declare_dram_parameter("bm2", [1, 1], F32, isOutput=False)
    idx16 = nc.declare_dram_parameter("idx16", [128, S // 16], I16, isOutput=False)
    dl16 = nc.declare_dram_parameter("dl16", [128, Ctot], F16, isOutput=False)
    segsel = nc.declare_dram_parameter("segsel", [NLOC, B], F16, isOutput=False)
    invcnt = nc.declare_dram_parameter("invcnt", [B, 1], F32, isOutput=False)
    iota128 = nc.declare_dram_parameter("iota128", [128, 256], F16, isOutput=False)
    ident64 = nc.declare_dram_parameter("ident64", [HID, HID], F16, isOutput=False)
    ident128 = nc.declare_dram_parameter("ident128", [128, 128], F16, isOutput=False)
    identB = nc.declare_dram_parameter("identB", [B, B], F32, isOutput=False)
    z = nc.declare_dram_parameter("z", [1, B], F32, isOutput=True)

    t_loc_q = [nc.dram_tensor(f"t_loc{q}", [QLOC, 128], F16)
               for q in range(NGRP)]
    # double-buffered by layer parity: layer l reads parity l%2 while layer
    # l+1's AllGathers (issued mid-edge-pass) write parity (l+1)%2
    t_full_q = [[nc.dram_tensor(f"t_full{p}_{q}", [GRP, 128], F16,
                                addr_space="Shared")
                 for q in range(NGRP)] for p in range(2)]
    pool_in = nc.dram_tensor("pool_in", [B, HID], F32)
    pool_out = nc.dram_tensor("pool_out", [B, HID], F32, addr_space="Shared")

    groups = [list(range(NCORES))]

    with tile.TileContext(nc) as tc:
        with (
            tc.tile_pool(name="const", bufs=1) as constp,
            tc.tile_pool(name="hT", bufs=1) as hTp,
            tc.tile_pool(name="tstage", bufs=2) as tstp,
            tc.tile_pool(name="xblk", bufs=2) as xblkp,
            tc.tile_pool(name="idx", bufs=3) as idxp,
            tc.tile_pool(name="msg", bufs=3) as msgp,
            tc.tile_pool(name="sel", bufs=2) as selp,
            tc.tile_pool(name="eps", bufs=3) as epsp,
            tc.tile_pool(name="psA", bufs=2, space="PSUM") as psA,
            tc.tile_pool(name="psB", bufs=2, space="PSUM") as psB,
            tc.tile_pool(name="psPool", bufs=1, space="PSUM") as psPoolp,
        ):
            disN_sb = constp.tile([128, NBLK], F32)
            nc.sync.dma_start(
                out=disN_sb[:], in_=disN.rearrange("(b p) o -> p (b o)", p=128))
            W0_sb = constp.tile([IN_C, HID], F16)
            nc.sync.dma_start(out=W0_sb[:], in_=W0[:])
            W1_sb = constp.tile([HID, HID], F16)
            nc.sync.dma_start(out=W1_sb[:], in_=W1[:])
            W2_sb = constp.tile([HID, HID], F16)
            nc.sync.dma_start(out=W2_sb[:], in_=W2[:])
            bias_sb = constp.tile([HID, 3], F32)
            for i, bb in enumerate([b0, b1, b2]):
                nc.sync.dma_start(out=bias_sb[:, i:i + 1], in_=bb[:])
            iota_sb = constp.tile([128, 256], F16)
            nc.sync.dma_start(out=iota_sb[:], in_=iota128[:])
            dl_sb = constp.tile([128, Ctot], F16)
            nc.sync.dma_start(out=dl_sb[:], in_=dl16[:])
            ident128_sb = constp.tile([128, 128], F16)
            nc.sync.dma_start(out=ident128_sb[:], in_=ident128[:])

            hT = hTp.tile([HID, NLOC], F16, tag="hT")
            pool_ps = psPoolp.tile([B, HID], F32)

            def table_mult(tstc, b, pt):
                nc.vector.tensor_tensor(
                    out=tstc[:, b, :], in0=pt[:],
                    in1=disN_sb[:, b:b + 1].to_broadcast([128, HID]),
                    op=mybir.AluOpType.mult)

            def stage_quarter(tstc, par, q):
                # duplicate the 64-wide table rows into 256B [t|t] rows via
                # two DMAs, then AllGather this slab into parity `par`
                tl = t_loc_q[q].rearrange("(b p) (h e) -> p b h e",
                                          p=128, h=2)
                src = tstc[:, q * QB:(q + 1) * QB, :].unsqueeze(2)
                for half in range(2):
                    nc.sync.dma_start(out=tl[:, :, half:half + 1, :], in_=src)
                nc.gpsimd.collective_compute(
                    "AllGather", mybir.AluOpType.bypass,
                    replica_groups=groups,
                    ins=[t_loc_q[q][:]], outs=[t_full_q[par][q][:]])

            # layer-0 table prologue (parity 0)
            tst0 = tstp.tile([128, NBLK, HID], F16, tag="tstage")
            tsts = [tst0]
            xq = None
            for b in range(NBLK):
                if b % QB == 0:
                    xq = xblkp.tile([IN_C, QLOC], F16, tag="xq")
                    nc.sync.dma_start(
                        out=xq[:],
                        in_=xT[:, (b // QB) * QLOC:(b // QB + 1) * QLOC])
                pt = psA.tile([128, HID], F32, tag="psA")
                nc.tensor.matmul(
                    pt[:], xq[:, (b % QB) * 128:(b % QB + 1) * 128],
                    W0_sb[:], start=True, stop=True)
                table_mult(tsts[0], b, pt)
                if (b + 1) % QB == 0:
                    stage_quarter(tsts[0], 0, b // QB)

            # late constants: not needed until layer 2 / the tail, so load
            # them behind the layer-0 edge pass instead of ahead of it
            ident64_sb = constp.tile([HID, HID], F16)
            nc.sync.dma_start(out=ident64_sb[:], in_=ident64[:])
            identB_sb = constp.tile([B, B], F32)
            nc.sync.dma_start(out=identB_sb[:], in_=identB[:])
            invcnt_sb = constp.tile([B, 1], F32)
            nc.sync.dma_start(out=invcnt_sb[:], in_=invcnt[:])
            mlpw_sb = constp.tile([HID, HID // 2 + 1], F32)
            nc.sync.dma_start(out=mlpw_sb[:, :HID // 2], in_=Wm1[:])
            nc.sync.dma_start(out=mlpw_sb[:HID // 2, HID // 2:], in_=Wm2[:])
            bm_sb = constp.tile([HID // 2, 2], F32)
            nc.sync.dma_start(out=bm_sb[:, 0:1], in_=bm1[:])
            nc.sync.dma_start(out=bm_sb[0:1, 1:2], in_=bm2[:])

            for l in range(3):
                par = l % 2
                tstc = tsts[l]
                if l < 2:
                    tstn = tstp.tile([128, NBLK, HID], F16, tag="tstage")
                    tsts.append(tstn)
                    Wn_sb = W1_sb if l == 0 else W2_sb

                for s in range(NSUP):
                    sup_slot0 = int(off_sup[s]) * 128
                    sup_slots = int(C_sup_g[s].sum()) * 128
                    disTB = idxp.tile([HID, SB * 128], F32, tag="disTB")
                    nc.sync.dma_start(
                        out=disTB[:].unsqueeze(1),
                        in_=disT[:, s * SB * 128:(s + 1) * SB * 128]
                        .partition_broadcast(HID))
                    idxb = idxp.tile([128, sup_slots // 16], I16, tag="idx")
                    nc.sync.dma_start(
                        out=idxb[:],
                        in_=idx16[:, sup_slot0 // 16:(sup_slot0 + sup_slots) // 16])
                    mts, selAs, selBs = [], [], []
                    goff = 0
                    for g in range(NGRPL):
                        cg = int(C[s, g])
                        ca, cb0 = int(cA[s, g]), int(cB0[s, g])
                        dl0 = int(off_sup[s]) + goff
                        selA = selp.tile([128, ca, 128], F16, tag=f"selA{g}")
                        nc.vector.tensor_tensor(
                            out=selA[:],
                            in0=dl_sb[:, dl0:dl0 + ca]
                            .unsqueeze(2).to_broadcast([128, ca, 128]),
                            in1=iota_sb[:, 0:128].unsqueeze(1)
                            .to_broadcast([128, ca, 128]),
                            op=mybir.AluOpType.is_equal)
                        nb = cg - cb0
                        selB = selp.tile([128, nb, 128], F16, tag=f"selB{g}")
                        nc.vector.tensor_tensor(
                            out=selB[:],
                            in0=dl_sb[:, dl0 + cb0:dl0 + cg]
                            .unsqueeze(2).to_broadcast([128, nb, 128]),
                            in1=iota_sb[:, 128:256].unsqueeze(1)
                            .to_broadcast([128, nb, 128]),
                            op=mybir.AluOpType.is_equal)
                        selAs.append(selA)
                        selBs.append(selB)
                        mt = msgp.tile([128, cg, 128], F16, tag=f"msg{g}")
                        nidx = cg * 128
                        nc.gpsimd.dma_gather(
                            out_ap=mt[:],
                            in_ap=t_full_q[par][g][:, :],
                            idxs_ap=idxb[:, (goff * 128) // 16:
                                         (goff * 128 + nidx) // 16],
                            num_idxs=nidx, num_idxs_reg=nidx,
                            elem_size=128,
                            single_packet=False, queue_num=g % NSWQ)
                        mts.append(mt)
                        goff += cg
                    for bi in range(SB):
                        b = s * SB + bi
                        agg = psB.tile([HID, 128], F32, tag="psB")
                        if bi == 0:
                            njob = int(cA[s].sum())
                        else:
                            njob = int((C[s] - cB0[s]).sum())
                        # seed with the self-loop term: agg = t_blk^T
                        # (epilogue's *dis makes it dis^2 * (hW), the
                        # reference's self-loop contribution)
                        nc.tensor.matmul(
                            agg[:], tstc[:, b, :], ident128_sb[:],
                            start=True, stop=False)
                        j = 0
                        for g in range(NGRPL):
                            if bi == 0:
                                rng_c = range(int(cA[s, g]))
                                stile, soff = selAs[g], 0
                            else:
                                rng_c = range(int(cB0[s, g]), int(C[s, g]))
                                stile, soff = selBs[g], int(cB0[s, g])
                            for c in rng_c:
                                nc.tensor.matmul(
                                    agg[:], mts[g][:, c, 0:HID],
                                    stile[:, c - soff, :],
                                    start=False, stop=(j == njob - 1))
                                j += 1
                        ep = epsp.tile([HID, 128], F32, tag="eps")
                        nc.vector.tensor_tensor(
                            out=ep[:], in0=agg[:],
                            in1=disTB[:, bi * 128:(bi + 1) * 128],
                            op=mybir.AluOpType.mult)
                        nc.scalar.activation(
                            out=hT[:, b * 128:(b + 1) * 128], in_=ep[:],
                            func=mybir.ActivationFunctionType.Relu,
                            bias=bias_sb[:, l:l + 1])
                        if l < 2:
                            # next layer's table block b, immediately after
                            # its hT lands; each finished quarter is staged
                            # and AllGathered into the other parity while
                            # this layer's edge pass continues
                            pt = psA.tile([128, HID], F32, tag="psA")
                            nc.tensor.matmul(
                                pt[:], hT[:, b * 128:(b + 1) * 128],
                                Wn_sb[:], start=True, stop=True)
                            table_mult(tstn, b, pt)
                            if (b + 1) % QB == 0:
                                stage_quarter(tstn, 1 - par, b // QB)
                        else:
                            # pooling interleaved into layer 2's edge pass;
                            # segsel streamed per quarter to save SBUF
                            if b % QB == 0:
                                segq = xblkp.tile([128, QB, B], F16,
                                                  tag="segq")
                                nc.sync.dma_start(
                                    out=segq[:],
                                    in_=segsel[(b // QB) * QLOC:
                                               (b // QB + 1) * QLOC, :]
                                    .rearrange("(b p) g -> p b g", p=128))
                            h3t = psA.tile([128, HID], F32, tag="psA")
                            nc.tensor.matmul(
                                h3t[:], hT[:, b * 128:(b + 1) * 128],
                                ident64_sb[:], start=True, stop=True)
                            h3s = epsp.tile([128, HID], F16, tag="h3s")
                            nc.vector.tensor_copy(out=h3s[:], in_=h3t[:])
                            nc.tensor.matmul(
                                pool_ps[:], segq[:, b % QB, :], h3s[:],
                                start=(b == 0), stop=(b == NBLK - 1))

            pool_sb = epsp.tile([B, HID], F32, tag="poolsb")
            nc.vector.tensor_copy(out=pool_sb[:], in_=pool_ps[:])
            nc.sync.dma_start(out=pool_in[:], in_=pool_sb[:])
            nc.gpsimd.collective_compute(
                "AllReduce", mybir.AluOpType.add, replica_groups=groups,
                ins=[pool_in[:]], outs=[pool_out[:]])
            pooled = epsp.tile([B, HID], F32, tag="pooled")
            nc.sync.dma_start(out=pooled[:], in_=pool_out[:])
            pm = epsp.tile([B, HID], F32, tag="pm")
            nc.vector.tensor_tensor(
                out=pm[:], in0=pooled[:],
                in1=invcnt_sb[:].to_broadcast([B, HID]),
                op=mybir.AluOpType.mult)
            ppT = psA.tile([HID, B], F32, tag="psA")
            nc.tensor.matmul(ppT[:], pm[:], identB_sb[:], start=True, stop=True)
            pT = epsp.tile([HID, B], F32, tag="pT")
            nc.vector.tensor_copy(out=pT[:], in_=ppT[:])
            z1p = psB.tile([HID // 2, B], F32, tag="psB")
            nc.tensor.matmul(z1p[:], mlpw_sb[:, :HID // 2], pT[:], start=True, stop=True)
            z1 = epsp.tile([HID // 2, B], F32, tag="z1")
            nc.scalar.activation(
                out=z1[:], in_=z1p[:],
                func=mybir.ActivationFunctionType.Relu, bias=bm_sb[:, 0:1])
            z2p = psB.tile([1, B], F32, tag="psB")
            nc.tensor.matmul(
                z2p[:], mlpw_sb[:HID // 2, HID // 2:HID // 2 + 1], z1[:],
                start=True, stop=True)
            zf = epsp.tile([1, B], F32, tag="zf")
            nc.vector.tensor_tensor(
                out=zf[:], in0=z2p[:],
                in1=bm_sb[0:1, 1:2].to_broadcast([1, B]),
                op=mybir.AluOpType.add)
            nc.sync.dma_start(out=z[:], in_=zf[:])

    nc.finalize()
    return nc


_CACHE = {}


def kernel(x, edge_index, batch, W0, b0, W1, b1, W2, b2, Wm1, bm1, Wm2, bm2,
           trace=False):
    x = np.asarray(x, np.float32)
    dis, Cm, idx_stream, dl_stream, perm = _preprocess(
        np.asarray(edge_index), np.asarray(batch))
    C = Cm[0]
    Ctot = int(C.sum())
    S = Ctot * 128

    gid = np.asarray(batch, np.int64)
    cnts = np.bincount(gid, minlength=B).astype(np.float32)
    packed = perm

    xp = np.zeros((NCORES, IN_C, NLOC), np.float16)
    xp[packed // NLOC, :, packed % NLOC] = x.astype(np.float16)
    disp = np.zeros((NCORES, NLOC), np.float32)
    disp[packed // NLOC, packed % NLOC] = dis
    seg = np.zeros((NCORES, NLOC, B), np.float16)
    seg[packed // NLOC, packed % NLOC, gid] = 1.0

    iota = np.tile(np.arange(256, dtype=np.float16)[None, :], (128, 1))
    in_maps = []
    for k in range(NCORES):
        in_maps.append(dict(
            xT=xp[k],
            disN=disp[k][:, None].copy(),
            disT=disp[k][None, :].copy(),
            W0=np.asarray(W0, np.float16),
            W1=np.asarray(W1, np.float16),
            W2=np.asarray(W2, np.float16),
            b0=np.asarray(b0, np.float32)[:, None],
            b1=np.asarray(b1, np.float32)[:, None],
            b2=np.asarray(b2, np.float32)[:, None],
            Wm1=np.asarray(Wm1, np.float32),
            bm1=np.asarray(bm1, np.float32)[:, None],
            Wm2=np.asarray(Wm2, np.float32),
            bm2=np.asarray(bm2, np.float32).reshape(1, 1),
            idx16=np.tile(idx_stream[k].reshape(S // 16, 16).T, (8, 1)).copy(),
            dl16=dl_stream[k].reshape(Ctot, 128).T.copy(),
            segsel=seg[k],
            invcnt=(1.0 / np.maximum(cnts, 1.0)).astype(np.float32)[:, None],
            iota128=iota,
            ident64=np.eye(HID, dtype=np.float16),
            ident128=np.eye(128, dtype=np.float16),
            identB=np.eye(B, dtype=np.float32),
        ))

    ckey = C.tobytes() + Cm[1].tobytes() + Cm[2].tobytes() + Cm[3].tobytes()
    if ckey not in _CACHE:
        _CACHE[ckey] = _build_nc(Cm)
    nc = _CACHE[ckey]

    res = run_bass_kernel_spmd(nc, in_maps, list(range(NCORES)), trace=trace)
    out = res.results[0]["z"].reshape(B, 1).astype(np.float32)
    if trace:
        return out, res
    return out
